# revision 10
# baseline (speedup 1.0000x reference)
"""Bass/Tile kernel for 2-layer edge-featured GAT (AblationGAT) on 8 trn2 cores.

Strategy (edge-parallel, dst-sharded):
  - Nodes padded to NP = n_cores * B * 128; core k owns blocks [k*B, (k+1)*B).
  - Host sorts edges by dst, assigns each edge to the core owning its dst
    block, splits per-block edges into lo/hi halves by src (int16 gather
    limit), pads each half to chunks of 128 edge slots (mask=0 pads).
  - Phase A (per core, redundant): full projection tables
      table1[n] = [h1(128) | a_src1(4) | a_dst1(4) | pad]   (f32, 768B rows)
    plus own1 = the same rows for the core's own nodes (fixed local offsets).
  - Layer pass (per dst block): dma_gather rows of table1 by src (4 SWDGE
    queues), build per-edge logits, segment-softmax numerators via exp
    (no max subtraction needed: |logit| is small), then scatter-add into
    a PSUM accumulator via one-hot selection matmuls. Self-loops (PyG
    fill_value='mean') handled in a per-block epilogue using deg / attr-sum
    columns accumulated in the same matmul.
  - AllGather of per-core layer-2 tables (h2 | a_src2 | a_dst2), then the
    same machinery for layer 2; output shard is the core's own node range.
"""

import math

import numpy as np

import concourse.bass as bass
import concourse.mybir as mybir
import concourse.tile as tile
from concourse import bacc
from concourse.masks import make_identity

F32 = mybir.dt.float32
I16 = mybir.dt.int16
I32 = mybir.dt.int32
AF = mybir.ActivationFunctionType
OP = mybir.AluOpType

NEG_SLOPE = 0.2
N_CORES = 8
CHUNK = 128          # edge slots per chunk (= PE contraction dim)
CALL_CHUNKS = 8      # chunks per dma_gather call (<= 1024 idx, HW limit)
ELEM1 = 192          # table1 row f32 elements (768B, %256B)
ELEM2 = 64           # table2 row f32 elements (256B)


def _ap3(ap, dims):
    """Raw AP with explicit [step, count] dims on the same tensor/offset."""
    return bass.AP(ap.tensor, ap.offset, [list(d) for d in dims])


def _bcast_last(ap, m):
    """[P, K] -> [P, K, m] with 0-stride last dim."""
    return _ap3(ap, list(ap.ap) + [[0, m]])


def _bcast_mid(ap, n):
    """[P, K] -> [P, n, K] with 0-stride middle dim."""
    return _ap3(ap, [list(ap.ap[0]), [0, n], list(ap.ap[-1])])


class Plan:
    pass


def host_prep(inputs, n_cores=N_CORES, half=32768):
    x = np.asarray(inputs["x"], np.float32)
    edge_idx = np.asarray(inputs["edge_idx"])
    edge_attr = np.asarray(inputs["edge_attr"], np.float32)[:, 0]
    W1 = np.asarray(inputs["W1"], np.float32)
    a_src1 = np.asarray(inputs["a_src1"], np.float32)
    a_dst1 = np.asarray(inputs["a_dst1"], np.float32)
    We1 = np.asarray(inputs["We1"], np.float32)
    a_e1 = np.asarray(inputs["a_e1"], np.float32)
    b1 = np.asarray(inputs["b1"], np.float32)
    W2 = np.asarray(inputs["W2"], np.float32)
    a_src2 = np.asarray(inputs["a_src2"], np.float32)
    a_dst2 = np.asarray(inputs["a_dst2"], np.float32)
    We2 = np.asarray(inputs["We2"], np.float32)
    a_e2 = np.asarray(inputs["a_e2"], np.float32)
    b2 = np.asarray(inputs["b2"], np.float32)

    p = Plan()
    N, IN_C = x.shape
    E = edge_idx.shape[1]
    HEADS, HID = a_src1.shape
    HC = HEADS * HID
    OUT_C = W2.shape[1]
    B = math.ceil(N / (128 * n_cores))     # blocks per core
    NPC = B * 128                          # nodes per core
    NP = NPC * n_cores                     # padded node count
    assert half % 128 == 0 and half <= NP
    p.N, p.E, p.IN_C, p.HEADS, p.HID, p.HC, p.OUT_C = N, E, IN_C, HEADS, HID, HC, OUT_C
    p.B, p.NPC, p.NP, p.n_cores, p.HALF = B, NPC, NP, n_cores, half

    x_pad = np.zeros((NP, IN_C), np.float32)
    x_pad[:N] = x

    src = edge_idx[0].astype(np.int64)
    dst = edge_idx[1].astype(np.int64)
    order = np.argsort(dst, kind="stable")
    src, dst, attr = src[order], dst[order], edge_attr[order]

    # per (core, block, half) edge lists
    counts = np.zeros((n_cores, B, 2), np.int64)
    edges = [[[None, None] for _ in range(B)] for _ in range(n_cores)]
    blk = dst // 128
    blk_starts = np.searchsorted(blk, np.arange(NP // 128 + 1))
    for g in range(NP // 128):
        k, b = divmod(g, B)
        lo_, hi_ = blk_starts[g], blk_starts[g + 1]
        s_, d_, a_ = src[lo_:hi_], dst[lo_:hi_], attr[lo_:hi_]
        m = s_ < half
        edges[k][b][0] = (s_[m], d_[m] - g * 128, a_[m])
        edges[k][b][1] = (s_[~m] - half, d_[~m] - g * 128, a_[~m])
        counts[k, b, 0] = m.sum()
        counts[k, b, 1] = (~m).sum()

    cn = np.maximum(np.ceil(counts / CHUNK).astype(np.int64).max(axis=0), 0)
    p.cnA = cn[:, 0].tolist()   # lo chunks per block slot (uniform across cores)
    p.cnB = cn[:, 1].tolist()
    TC = int(sum(p.cnA) + sum(p.cnB))   # total chunks per core
    p.TC = TC

    gidx = np.zeros((n_cores, 16, TC * 8), np.int16)
    dstrel = np.zeros((n_cores, 128, TC), np.float32)
    maskattr = np.zeros((n_cores, 128, TC * 2), np.float32)
    for k in range(n_cores):
        c0 = 0
        for b in range(B):
            for hf in range(2):
                nch = (p.cnA[b], p.cnB[b])[hf]
                if nch == 0:
                    continue
                s_, dr_, a_ = edges[k][b][hf]
                ne = len(s_)
                nslots = nch * CHUNK
                sv = np.zeros(nslots, np.int16)
                dv = np.zeros(nslots, np.float32)
                av = np.zeros(nslots, np.float32)
                mv = np.zeros(nslots, np.float32)
                sv[:ne] = s_
                dv[:ne] = dr_
                av[:ne] = a_
                mv[:ne] = 1.0
                for c in range(nch):
                    sl = slice(c * CHUNK, (c + 1) * CHUNK)
                    cc = c0 + c
                    # gather slot i -> idx[i % 16, 8*chunk + i // 16]
                    gidx[k, :, cc * 8 : cc * 8 + 8] = sv[sl].reshape(8, 16).T
                    dstrel[k, :, cc] = dv[sl]
                    maskattr[k, :, cc * 2] = mv[sl]
                    maskattr[k, :, cc * 2 + 1] = av[sl]
                c0 += nch
        assert c0 == TC

    # weights packaging
    WaWd = np.zeros((HC, 2 * HEADS), np.float32)
    for h in range(HEADS):
        WaWd[h * HID : (h + 1) * HID, h] = a_src1[h]
        WaWd[h * HID : (h + 1) * HID, HEADS + h] = a_dst1[h]

    p.in_maps = []
    for k in range(n_cores):
        p.in_maps.append(
            {
                "x": x_pad,
                "x_own": x_pad[k * NPC : (k + 1) * NPC].copy(),
                "W1": W1,
                "WaWd": WaWd,
                "W2": W2,
                "b1row": b1[None, :],
                "b2row": b2[None, :],
                "We1row": We1,
                "ae1row": a_e1.reshape(1, HC),
                "We2row": We2,
                "ae2row": a_e2.reshape(1, OUT_C),
                "asrc2row": a_src2,
                "adst2row": a_dst2,
                "gidx": np.tile(gidx[k], (8, 1)),
                "dstrel": dstrel[k],
                "maskattr": maskattr[k],
            }
        )
    return p


def build(p):
    nc = bacc.Bacc(
        "TRN2",
        target_bir_lowering=False,
        debug=False,
        num_devices=p.n_cores,
        num_swdge_queues=4,
    )
    IN_C, HC, OUT_C, HEADS, HID = p.IN_C, p.HC, p.OUT_C, p.HEADS, p.HID
    NP, NPC, B, TC, HALF = p.NP, p.NPC, p.B, p.TC, p.HALF

    x_ext = nc.dram_tensor("x", [NP, IN_C], F32, kind="ExternalInput")
    xown_ext = nc.dram_tensor("x_own", [NPC, IN_C], F32, kind="ExternalInput")
    W1_ext = nc.dram_tensor("W1", [IN_C, HC], F32, kind="ExternalInput")
    WaWd_ext = nc.dram_tensor("WaWd", [HC, 2 * HEADS], F32, kind="ExternalInput")
    W2_ext = nc.dram_tensor("W2", [HC, OUT_C], F32, kind="ExternalInput")
    b1row_ext = nc.dram_tensor("b1row", [1, HC], F32, kind="ExternalInput")
    b2row_ext = nc.dram_tensor("b2row", [1, OUT_C], F32, kind="ExternalInput")
    We1row_ext = nc.dram_tensor("We1row", [1, HC], F32, kind="ExternalInput")
    ae1row_ext = nc.dram_tensor("ae1row", [1, HC], F32, kind="ExternalInput")
    We2row_ext = nc.dram_tensor("We2row", [1, OUT_C], F32, kind="ExternalInput")
    ae2row_ext = nc.dram_tensor("ae2row", [1, OUT_C], F32, kind="ExternalInput")
    asrc2row_ext = nc.dram_tensor("asrc2row", [1, OUT_C], F32, kind="ExternalInput")
    adst2row_ext = nc.dram_tensor("adst2row", [1, OUT_C], F32, kind="ExternalInput")
    gidx_ext = nc.dram_tensor("gidx", [128, TC * 8], I16, kind="ExternalInput")
    dstrel_ext = nc.dram_tensor("dstrel", [128, TC], F32, kind="ExternalInput")
    maskattr_ext = nc.dram_tensor("maskattr", [128, TC * 2], F32, kind="ExternalInput")
    out_ext = nc.dram_tensor("out", [NPC, OUT_C], F32, kind="ExternalOutput")

    table1 = nc.dram_tensor("table1", [NP, ELEM1], F32)
    own1 = nc.dram_tensor("own1", [NPC, IN_C + 8], F32)
    ag_in = nc.dram_tensor("ag_in", [NPC, ELEM2], F32)
    table2 = nc.dram_tensor("table2", [NP, ELEM2], F32, addr_space="Shared")

    C1 = IN_C           # table1 col: h1 at 0:HC(=128), asrc at 128:132, adst 132:136
    qn = [0]            # rotating gather queue

    def next_q():
        q = qn[0] % 4
        qn[0] += 1
        return q

    with tile.TileContext(nc) as tc:
        with (
            tc.tile_pool(name="consts", bufs=1) as cp,
            tc.tile_pool(name="streams", bufs=1) as strp,
        ):
            # ---------- constants & streams ----------
            ident = cp.tile([128, 128], F32)
            make_identity(nc, ident[:])
            iota_i = cp.tile([128, 128], I32)
            nc.gpsimd.iota(iota_i[:], pattern=[[1, 128]], base=0, channel_multiplier=0)
            iota_mat = cp.tile([128, 128], F32)
            nc.vector.tensor_copy(out=iota_mat[:], in_=iota_i[:])
            ones1 = cp.tile([1, 128], F32)
            nc.gpsimd.memset(ones1[:], 1.0)
            zeros_pad = cp.tile([128, ELEM2], F32)
            nc.gpsimd.memset(zeros_pad[:], 0.0)

            W1s = cp.tile([128, HC], F32)
            nc.sync.dma_start(out=W1s[:], in_=W1_ext[:, :])
            WaWds = cp.tile([128, 2 * HEADS], F32)
            nc.sync.dma_start(out=WaWds[:], in_=WaWd_ext[:, :])
            W2s = cp.tile([128, OUT_C], F32)
            nc.sync.dma_start(out=W2s[:], in_=W2_ext[:, :])

            rows = cp.tile([1, 128], F32, tag="rows")
            gidx_t = strp.tile([128, TC * 8], I16)
            nc.sync.dma_start(out=gidx_t[:], in_=gidx_ext[:, :])
            dstrel_t = strp.tile([128, TC], F32)
            nc.sync.dma_start(out=dstrel_t[:], in_=dstrel_ext[:, :])
            maskattr_t = strp.tile([128, TC * 2], F32)
            nc.sync.dma_start(out=maskattr_t[:], in_=maskattr_ext[:, :])

            la_keep = strp.tile([128, B], F32)

            # broadcast helper: row [1, n] -> [128, n] via ones-matmul
            with tc.tile_pool(name="bc_psum", bufs=2, space="PSUM") as bps:

                def bcast_row(row_ap, n, out_tile):
                    ps = bps.tile([128, 512], F32, tag="bc")
                    nc.tensor.matmul(
                        out=ps[:, :n], lhsT=ones1[:], rhs=row_ap, start=True, stop=True
                    )
                    nc.vector.tensor_copy(out=out_tile[:], in_=ps[:, :n])

                # c1[h] = sum_c We1[h*HID+c] * ae1[h*HID+c]; c_mat [128, HEADS]
                t_we = cp.tile([1, HC], F32, tag="t_we")
                nc.sync.dma_start(out=t_we[:], in_=We1row_ext[:, :])
                t_ae = cp.tile([1, HC], F32, tag="t_ae")
                nc.sync.dma_start(out=t_ae[:], in_=ae1row_ext[:, :])
                t_pr = cp.tile([1, HC], F32, tag="t_pr")
                nc.vector.tensor_mul(out=t_pr[:], in0=t_we[:], in1=t_ae[:])
                c1row = cp.tile([1, HEADS], F32, tag="c1row")
                nc.vector.tensor_reduce(
                    out=c1row[:],
                    in_=t_pr[:].rearrange("p (h c) -> p h c", c=HID),
                    axis=mybir.AxisListType.X,
                    op=OP.add,
                )
                c_mat = cp.tile([128, HEADS], F32)
                bcast_row(c1row[:], HEADS, c_mat)

                # c2 scalar -> c2col [128, 1]
                t_we2 = cp.tile([1, OUT_C], F32, tag="t_we2")
                nc.sync.dma_start(out=t_we2[:], in_=We2row_ext[:, :])
                t_ae2 = cp.tile([1, OUT_C], F32, tag="t_ae2")
                nc.sync.dma_start(out=t_ae2[:], in_=ae2row_ext[:, :])
                t_pr2 = cp.tile([1, OUT_C], F32, tag="t_pr2")
                nc.vector.tensor_mul(out=t_pr2[:], in0=t_we2[:], in1=t_ae2[:])
                c2row = cp.tile([1, 1], F32, tag="c2row")
                nc.vector.tensor_reduce(
                    out=c2row[:], in_=t_pr2[:], axis=mybir.AxisListType.X, op=OP.add
                )
                c2col = cp.tile([128, 1], F32)
                bcast_row(c2row[:], 1, c2col)

                b1mat = cp.tile([128, HC], F32)
                nc.sync.dma_start(out=rows[:, :HC], in_=b1row_ext[:, :])
                bcast_row(rows[:, :HC], HC, b1mat)
                b2mat = cp.tile([128, OUT_C], F32)
                rows2 = cp.tile([1, OUT_C], F32, tag="rows2")
                nc.sync.dma_start(out=rows2[:], in_=b2row_ext[:, :])
                bcast_row(rows2[:], OUT_C, b2mat)
                asrc2w = cp.tile([128, OUT_C], F32)
                rows3 = cp.tile([1, OUT_C], F32, tag="rows3")
                nc.sync.dma_start(out=rows3[:], in_=asrc2row_ext[:, :])
                bcast_row(rows3[:], OUT_C, asrc2w)
                adst2w = cp.tile([128, OUT_C], F32)
                rows4 = cp.tile([1, OUT_C], F32, tag="rows4")
                nc.sync.dma_start(out=rows4[:], in_=adst2row_ext[:, :])
                bcast_row(rows4[:], OUT_C, adst2w)

            # ---------- phase A: projection tables ----------
            def proj_rows(src_dram, g, dst_dram, dst_off):
                """rows [g*128, (g+1)*128) of src_dram -> table rows in dst_dram."""
                xa = pA["x"].tile([128, IN_C], F32, tag="xa")
                nc.sync.dma_start(
                    out=xa[:], in_=src_dram[g * 128 : (g + 1) * 128, :]
                )
                pt = pA["ps"].tile([128, 128], F32, space="PSUM", tag="pt")
                nc.tensor.transpose(out=pt[:, :IN_C], in_=xa[:], identity=ident[:])
                xT = pA["x"].tile([128, 128], F32, tag="xT")
                nc.vector.tensor_copy(out=xT[:, :], in_=pt[:, :])
                ph = pA["ps"].tile([128, HC], F32, space="PSUM", tag="ph")
                nc.tensor.matmul(
                    out=ph[:], lhsT=xT[:IN_C, :], rhs=W1s[:IN_C, :], start=True, stop=True
                )
                row_w = dst_dram.shape[1]
                row = pA["x"].tile([128, row_w], F32, tag="row")
                nc.vector.tensor_copy(out=row[:, 0:HC], in_=ph[:])
                pt2 = pA["ps"].tile([128, 128], F32, space="PSUM", tag="pt")
                nc.tensor.transpose(
                    out=pt2[:, :HC], in_=row[:, 0:HC], identity=ident[:]
                )
                h1T = pA["x"].tile([128, 128], F32, tag="h1T")
                nc.vector.tensor_copy(out=h1T[:], in_=pt2[:])
                paw = pA["ps"].tile([128, 2 * HEADS], F32, space="PSUM", tag="ph")
                nc.tensor.matmul(
                    out=paw[:], lhsT=h1T[:HC, :], rhs=WaWds[:HC, :], start=True, stop=True
                )
                nc.vector.tensor_copy(
                    out=row[:, HC : HC + 2 * HEADS], in_=paw[:]
                )
                if row_w > HC + 2 * HEADS:
                    nc.gpsimd.memset(row[:, HC + 2 * HEADS : row_w], 0.0)
                nc.sync.dma_start(
                    out=dst_dram[dst_off + g * 128 : dst_off + (g + 1) * 128, :],
                    in_=row[:],
                )

            with (
                tc.tile_pool(name="pA_x", bufs=3) as pA_x,
                tc.tile_pool(name="pA_ps", bufs=2, space="PSUM") as pA_ps,
            ):
                pA = {"x": pA_x, "ps": pA_ps}
                for g in range(NP // 128):
                    proj_rows(x_ext, g, table1, 0)
                for b in range(B):
                    proj_rows(xown_ext, b, own1, 0)

            # ---------- shared per-layer machinery ----------
            def layer_pass(
                layer,
                table_dram,
                elem,
                nheads,
                msg_w,      # message width (= nheads * per-head channels)
                ob_dram,    # per-core own rows [NPC, *]: h|asrc|adst columns
                ob_cols,    # (h_lo, h_hi, asrc_lo, asrc_hi, adst_lo, adst_hi)
                cvec,       # [128, nheads] (c_mat or c2col)
                do_degattr, # accumulate deg/attr columns (layer 1)
                epilogue,   # fn(b, P, OB, exl) -> None
            ):
                hlo, hhi, alo, ahi, dlo, dhi = ob_cols
                rhs_w = msg_w + nheads + (2 if do_degattr else 0)
                c_glob = [0]
                with (
                    tc.tile_pool(name=f"g{layer}", bufs=3) as gp,
                    tc.tile_pool(name=f"rhs{layer}", bufs=3) as rhp,
                    tc.tile_pool(name=f"u{layer}", bufs=3) as upo,
                    tc.tile_pool(name=f"sel{layer}", bufs=4) as selp,
                    tc.tile_pool(name=f"sm{layer}", bufs=6) as smp,
                    tc.tile_pool(name=f"ob{layer}", bufs=2) as obp,
                    tc.tile_pool(name=f"ep{layer}", bufs=2) as epp,
                    tc.tile_pool(name=f"ps{layer}", bufs=2, space="PSUM") as psb,
                    tc.tile_pool(name=f"pst{layer}", bufs=2, space="PSUM") as pst,
                    tc.tile_pool(name=f"pss{layer}", bufs=2, space="PSUM") as pss,
                ):
                    for b in range(B):
                        OB = obp.tile([128, dhi], F32, tag="OB")
                        nc.sync.dma_start(
                            out=OB[:], in_=ob_dram[b * 128 : (b + 1) * 128, 0:dhi]
                        )
                        ncht = p.cnA[b] + p.cnB[b]
                        if ncht > 0:
                            pblk = psb.tile(
                                [128, rhs_w], F32, space="PSUM", tag="pblk"
                            )
                        cdone = 0
                        for hf in range(2):
                            nch = (p.cnA[b], p.cnB[b])[hf]
                            if nch == 0:
                                continue
                            tbl = (
                                table_dram[0:HALF, :]
                                if hf == 0
                                else table_dram[HALF:NP, :]
                            )
                            for c0 in range(0, nch, CALL_CHUNKS):
                                ncall = min(CALL_CHUNKS, nch - c0)
                                cg0 = c_glob[0] + cdone + c0
                                GA = gp.tile([128, CALL_CHUNKS * elem], F32, tag="GA")
                                nc.gpsimd.dma_gather(
                                    out_ap=GA[:, : ncall * elem].rearrange(
                                        "p (n k) -> p n k", k=elem
                                    ),
                                    in_ap=tbl,
                                    idxs_ap=gidx_t[:, cg0 * 8 : (cg0 + ncall) * 8],
                                    num_idxs=ncall * CHUNK,
                                    num_idxs_reg=ncall * CHUNK,
                                    elem_size=elem,
                                    queue_num=next_q(),
                                )
                                # u = attr*c + asrc_gathered  [128, ncall, nheads]
                                u = upo.tile(
                                    [128, CALL_CHUNKS * nheads], F32, tag="u"
                                )
                                u3 = u[:, : ncall * nheads].rearrange(
                                    "p (n k) -> p n k", k=nheads
                                )
                                attr_sl = _ap3(
                                    maskattr_t[:, cg0 * 2 + 1 : cg0 * 2 + 2],
                                    [
                                        list(maskattr_t[:].ap[0]),
                                        [2, ncall],
                                        [0, nheads],
                                    ],
                                )
                                nc.vector.tensor_tensor(
                                    out=u3,
                                    in0=attr_sl,
                                    in1=_bcast_mid(cvec[:], ncall),
                                    op=OP.mult,
                                )
                                ga_asrc0 = GA[:, msg_w : msg_w + nheads]
                                nc.vector.tensor_tensor(
                                    out=u3,
                                    in0=u3,
                                    in1=_ap3(
                                        ga_asrc0,
                                        [
                                            list(ga_asrc0.ap[0]),
                                            [elem, ncall],
                                            [1, nheads],
                                        ],
                                    ),
                                    op=OP.add,
                                )
                                rhs = rhp.tile(
                                    [128, CALL_CHUNKS * rhs_w], F32, tag="rhs"
                                )
                                if do_degattr:
                                    rh_ma0 = rhs[
                                        :, msg_w + nheads : msg_w + nheads + 2
                                    ]
                                    ma0 = maskattr_t[:, cg0 * 2 : cg0 * 2 + 2]
                                    nc.vector.tensor_copy(
                                        out=_ap3(
                                            rh_ma0,
                                            [
                                                list(rh_ma0.ap[0]),
                                                [rhs_w, ncall],
                                                [1, 2],
                                            ],
                                        ),
                                        in_=_ap3(
                                            ma0,
                                            [
                                                list(ma0.ap[0]),
                                                [2, ncall],
                                                [1, 2],
                                            ],
                                        ),
                                    )
                                for c in range(ncall):
                                    cg = cg0 + c
                                    is_first = cdone + c0 + c == 0
                                    is_last = cdone + c0 + c == ncht - 1
                                    S = selp.tile([128, 128], F32, tag="S")
                                    nc.vector.tensor_tensor(
                                        out=S[:],
                                        in0=iota_mat[:],
                                        in1=dstrel_t[:, cg : cg + 1].to_broadcast(
                                            [128, 128]
                                        ),
                                        op=OP.is_equal,
                                    )
                                    pT = pst.tile(
                                        [128, 128], F32, space="PSUM", tag="pT"
                                    )
                                    nc.tensor.transpose(
                                        out=pT[:], in_=S[:], identity=ident[:]
                                    )
                                    ST = selp.tile([128, 128], F32, tag="ST")
                                    nc.vector.tensor_copy(out=ST[:], in_=pT[:])
                                    padst = pss.tile(
                                        [128, nheads], F32, space="PSUM", tag="padst"
                                    )
                                    nc.tensor.matmul(
                                        out=padst[:],
                                        lhsT=ST[:],
                                        rhs=OB[:, dlo:dhi],
                                        start=True,
                                        stop=True,
                                    )
                                    lg = smp.tile([128, nheads], F32, tag="lg")
                                    nc.vector.tensor_tensor(
                                        out=lg[:],
                                        in0=u[:, c * nheads : (c + 1) * nheads],
                                        in1=padst[:],
                                        op=OP.add,
                                    )
                                    lk = smp.tile([128, nheads], F32, tag="lk")
                                    nc.vector.scalar_tensor_tensor(
                                        out=lk[:], in0=lg[:], scalar=NEG_SLOPE,
                                        in1=lg[:], op0=OP.mult, op1=OP.max,
                                    )
                                    exf = smp.tile([128, nheads], F32, tag="exf")
                                    nc.scalar.activation(
                                        out=exf[:], in_=lk[:], func=AF.Exp
                                    )
                                    ex_dst = rhs[
                                        :, c * rhs_w + msg_w : c * rhs_w + msg_w + nheads
                                    ]
                                    nc.vector.tensor_tensor(
                                        out=ex_dst,
                                        in0=exf[:],
                                        in1=maskattr_t[
                                            :, cg * 2 : cg * 2 + 1
                                        ].to_broadcast([128, nheads]),
                                        op=OP.mult,
                                    )
                                    hw = msg_w // nheads
                                    nc.vector.tensor_tensor(
                                        out=rhs[
                                            :, c * rhs_w : c * rhs_w + msg_w
                                        ].rearrange("p (h w) -> p h w", w=hw),
                                        in0=GA[
                                            :, c * elem : c * elem + msg_w
                                        ].rearrange("p (h w) -> p h w", w=hw),
                                        in1=_bcast_last(ex_dst, hw),
                                        op=OP.mult,
                                    )
                                    nc.tensor.matmul(
                                        out=pblk[:],
                                        lhsT=S[:],
                                        rhs=rhs[:, c * rhs_w : (c + 1) * rhs_w],
                                        start=is_first,
                                        stop=is_last,
                                    )
                            cdone += nch
                        c_glob[0] += ncht

                        # ----- block epilogue -----
                        P = epp.tile([128, rhs_w], F32, tag="P")
                        if ncht == 0:
                            nc.gpsimd.memset(P[:], 0.0)
                        else:
                            nc.vector.tensor_copy(out=P[:], in_=pblk[:])
                        # self-loop logits
                        if do_degattr:
                            dm = epp.tile([128, 1], F32, tag="dm")
                            nc.vector.tensor_scalar_max(
                                out=dm[:], in0=P[:, msg_w + nheads : msg_w + nheads + 1],
                                scalar1=1.0,
                            )
                            rc = epp.tile([128, 1], F32, tag="rc")
                            nc.vector.reciprocal(out=rc[:], in_=dm[:])
                            nc.vector.tensor_tensor(
                                out=la_keep[:, b : b + 1],
                                in0=P[:, msg_w + nheads + 1 : msg_w + nheads + 2],
                                in1=rc[:],
                                op=OP.mult,
                            )
                        tl = epp.tile([128, nheads], F32, tag="tl")
                        nc.vector.tensor_tensor(
                            out=tl[:],
                            in0=la_keep[:, b : b + 1].to_broadcast([128, nheads]),
                            in1=cvec[:],
                            op=OP.mult,
                        )
                        nc.vector.tensor_tensor(
                            out=tl[:], in0=tl[:], in1=OB[:, alo:ahi], op=OP.add
                        )
                        nc.vector.tensor_tensor(
                            out=tl[:], in0=tl[:], in1=OB[:, dlo:dhi], op=OP.add
                        )
                        lkl = epp.tile([128, nheads], F32, tag="lkl")
                        nc.vector.scalar_tensor_tensor(
                            out=lkl[:], in0=tl[:], scalar=NEG_SLOPE,
                            in1=tl[:], op0=OP.mult, op1=OP.max,
                        )
                        exl = epp.tile([128, nheads], F32, tag="exl")
                        nc.scalar.activation(out=exl[:], in_=lkl[:], func=AF.Exp)
                        hw = msg_w // nheads
                        ml = epp.tile([128, msg_w], F32, tag="ml")
                        nc.vector.tensor_tensor(
                            out=ml[:].rearrange("p (h w) -> p h w", w=hw),
                            in0=OB[:, hlo:hhi].rearrange("p (h w) -> p h w", w=hw),
                            in1=_bcast_last(exl[:], hw),
                            op=OP.mult,
                        )
                        tm = epp.tile([128, msg_w], F32, tag="tm")
                        nc.vector.tensor_tensor(
                            out=tm[:], in0=P[:, 0:msg_w], in1=ml[:], op=OP.add
                        )
                        st = epp.tile([128, nheads], F32, tag="st")
                        nc.vector.tensor_tensor(
                            out=st[:],
                            in0=P[:, msg_w : msg_w + nheads],
                            in1=exl[:],
                            op=OP.add,
                        )
                        rs = epp.tile([128, nheads], F32, tag="rs")
                        nc.vector.reciprocal(out=rs[:], in_=st[:])
                        o1 = epp.tile([128, msg_w], F32, tag="o1")
                        nc.vector.tensor_tensor(
                            out=o1[:].rearrange("p (h w) -> p h w", w=hw),
                            in0=tm[:].rearrange("p (h w) -> p h w", w=hw),
                            in1=_bcast_last(rs[:], hw),
                            op=OP.mult,
                        )
                        epilogue(b, o1)

            # ---------- layer 1 ----------
            with (
                tc.tile_pool(name="e1", bufs=2) as e1p,
                tc.tile_pool(name="e1ps", bufs=1, space="PSUM") as e1ps,
            ):

                def epi1(b, o1):
                    # o1 = aggregated/normalized messages [128, HC] (pre-bias)
                    ob1 = e1p.tile([128, p.HC], F32, tag="ob1")
                    nc.vector.tensor_tensor(
                        out=ob1[:], in0=o1[:], in1=b1mat[:], op=OP.add
                    )
                    # elu
                    mn = e1p.tile([128, p.HC], F32, tag="mn")
                    nc.vector.tensor_scalar_min(out=mn[:], in0=ob1[:], scalar1=0.0)
                    em = e1p.tile([128, p.HC], F32, tag="em")
                    nc.scalar.activation(out=em[:], in_=mn[:], func=AF.Exp)
                    mx = e1p.tile([128, p.HC], F32, tag="mx")
                    nc.vector.tensor_scalar_max(out=mx[:], in0=ob1[:], scalar1=0.0)
                    x2 = e1p.tile([128, p.HC], F32, tag="x2")
                    nc.vector.tensor_add(out=x2[:], in0=mx[:], in1=em[:])
                    nc.vector.tensor_scalar_add(out=x2[:], in0=x2[:], scalar1=-1.0)
                    # h2 = x2 @ W2
                    pt = e1ps.tile([128, 128], F32, space="PSUM", tag="ept")
                    nc.tensor.transpose(out=pt[:, : p.HC], in_=x2[:], identity=ident[:])
                    x2T = e1p.tile([128, 128], F32, tag="x2T")
                    nc.vector.tensor_copy(out=x2T[:], in_=pt[:])
                    ph2 = e1ps.tile([128, p.OUT_C], F32, space="PSUM", tag="eph2")
                    nc.tensor.matmul(
                        out=ph2[:], lhsT=x2T[: p.HC, :], rhs=W2s[: p.HC, :],
                        start=True, stop=True,
                    )
                    h2s = e1p.tile([128, p.OUT_C], F32, tag="h2s")
                    nc.vector.tensor_copy(out=h2s[:], in_=ph2[:])
                    th = e1p.tile([128, p.OUT_C], F32, tag="th")
                    nc.vector.tensor_mul(out=th[:], in0=h2s[:], in1=asrc2w[:])
                    as2 = e1p.tile([128, 1], F32, tag="as2")
                    nc.vector.tensor_reduce(
                        out=as2[:], in_=th[:], axis=mybir.AxisListType.X, op=OP.add
                    )
                    nc.vector.tensor_mul(out=th[:], in0=h2s[:], in1=adst2w[:])
                    ad2 = e1p.tile([128, 1], F32, tag="ad2")
                    nc.vector.tensor_reduce(
                        out=ad2[:], in_=th[:], axis=mybir.AxisListType.X, op=OP.add
                    )
                    t2t = e1p.tile([128, ELEM2], F32, tag="t2t")
                    nc.vector.tensor_copy(out=t2t[:, : p.OUT_C], in_=h2s[:])
                    nc.vector.tensor_copy(
                        out=t2t[:, p.OUT_C : p.OUT_C + 1], in_=as2[:]
                    )
                    nc.vector.tensor_copy(
                        out=t2t[:, p.OUT_C + 1 : p.OUT_C + 2], in_=ad2[:]
                    )
                    nc.vector.tensor_copy(
                        out=t2t[:, p.OUT_C + 2 : ELEM2],
                        in_=zeros_pad[:, : ELEM2 - p.OUT_C - 2],
                    )
                    nc.sync.dma_start(
                        out=ag_in[b * 128 : (b + 1) * 128, :], in_=t2t[:]
                    )

                layer_pass(
                    layer=1,
                    table_dram=table1,
                    elem=ELEM1,
                    nheads=HEADS,
                    msg_w=HC,
                    ob_dram=own1,
                    ob_cols=(0, HC, HC, HC + HEADS, HC + HEADS, HC + 2 * HEADS),
                    cvec=c_mat,
                    do_degattr=True,
                    epilogue=epi1,
                )

            # ---------- allgather ----------
            nc.gpsimd.collective_compute(
                "AllGather",
                OP.bypass,
                replica_groups=[list(range(p.n_cores))],
                ins=[ag_in.ap().opt()],
                outs=[table2.ap().opt()],
            )

            # ---------- layer 2 ----------
            with tc.tile_pool(name="e2", bufs=2) as e2p:

                def epi2(b, o2):
                    ob2 = e2p.tile([128, p.OUT_C], F32, tag="ob2")
                    nc.vector.tensor_tensor(
                        out=ob2[:], in0=o2[:], in1=b2mat[:], op=OP.add
                    )
                    nc.sync.dma_start(
                        out=out_ext[b * 128 : (b + 1) * 128, :], in_=ob2[:]
                    )

                layer_pass(
                    layer=2,
                    table_dram=table2,
                    elem=ELEM2,
                    nheads=1,
                    msg_w=OUT_C,
                    ob_dram=ag_in,
                    ob_cols=(0, OUT_C, OUT_C, OUT_C + 1, OUT_C + 1, OUT_C + 2),
                    cvec=c2col,
                    do_degattr=False,
                    epilogue=epi2,
                )

    nc.compile()
    return nc


def kernel(**inputs):
    p = host_prep(inputs)
    nc = build(p)
    from concourse.bass_utils import run_bass_kernel_spmd

    res = run_bass_kernel_spmd(nc, p.in_maps, list(range(p.n_cores))).results
    out = np.concatenate([res[k]["out"] for k in range(p.n_cores)], axis=0)
    return out[: p.N]


# revision 12
# speedup vs baseline: 1.5741x; 1.5741x over previous
"""Bass/Tile kernel for 2-layer edge-featured GAT (AblationGAT) on 8 trn2 cores.

Strategy (edge-parallel, dst-sharded):
  - Nodes padded to NP = n_cores * B * 128; core k owns blocks [k*B, (k+1)*B).
  - Host sorts edges by dst, assigns each edge to the core owning its dst
    block, splits per-block edges into lo/hi halves by src (int16 gather
    limit), pads each half to chunks of 128 edge slots (mask=0 pads).
  - Phase A (per core, redundant): full projection tables
      table1[n] = [h1(128) | a_src1(4) | a_dst1(4) | pad]   (f32, 768B rows)
    plus own1 = the same rows for the core's own nodes (fixed local offsets).
  - Layer pass (per dst block): dma_gather rows of table1 by src (4 SWDGE
    queues), build per-edge logits, segment-softmax numerators via exp
    (no max subtraction needed: |logit| is small), then scatter-add into
    a PSUM accumulator via one-hot selection matmuls. Self-loops (PyG
    fill_value='mean') handled in a per-block epilogue using deg / attr-sum
    columns accumulated in the same matmul.
  - AllGather of per-core layer-2 tables (h2 | a_src2 | a_dst2), then the
    same machinery for layer 2; output shard is the core's own node range.
"""

import math

import numpy as np

import concourse.bass as bass
import concourse.mybir as mybir
import concourse.tile as tile
from concourse import bacc
from concourse.masks import make_identity

F32 = mybir.dt.float32
I16 = mybir.dt.int16
I32 = mybir.dt.int32
AF = mybir.ActivationFunctionType
OP = mybir.AluOpType

NEG_SLOPE = 0.2
N_CORES = 8
CHUNK = 128          # edge slots per chunk (= PE contraction dim)
CALL_CHUNKS = 8      # chunks per dma_gather call (<= 1024 idx, HW limit)
ELEM1 = 192          # table1 row f32 elements (768B, %256B)
ELEM2 = 64           # table2 row f32 elements (256B)


def _ap3(ap, dims):
    """Raw AP with explicit [step, count] dims on the same tensor/offset."""
    return bass.AP(ap.tensor, ap.offset, [list(d) for d in dims])


def _bcast_last(ap, m):
    """[P, K] -> [P, K, m] with 0-stride last dim."""
    return _ap3(ap, list(ap.ap) + [[0, m]])


def _bcast_mid(ap, n):
    """[P, K] -> [P, n, K] with 0-stride middle dim."""
    return _ap3(ap, [list(ap.ap[0]), [0, n], list(ap.ap[-1])])


class Plan:
    pass


def host_prep(inputs, n_cores=N_CORES, half=32768):
    x = np.asarray(inputs["x"], np.float32)
    edge_idx = np.asarray(inputs["edge_idx"])
    edge_attr = np.asarray(inputs["edge_attr"], np.float32)[:, 0]
    W1 = np.asarray(inputs["W1"], np.float32)
    a_src1 = np.asarray(inputs["a_src1"], np.float32)
    a_dst1 = np.asarray(inputs["a_dst1"], np.float32)
    We1 = np.asarray(inputs["We1"], np.float32)
    a_e1 = np.asarray(inputs["a_e1"], np.float32)
    b1 = np.asarray(inputs["b1"], np.float32)
    W2 = np.asarray(inputs["W2"], np.float32)
    a_src2 = np.asarray(inputs["a_src2"], np.float32)
    a_dst2 = np.asarray(inputs["a_dst2"], np.float32)
    We2 = np.asarray(inputs["We2"], np.float32)
    a_e2 = np.asarray(inputs["a_e2"], np.float32)
    b2 = np.asarray(inputs["b2"], np.float32)

    p = Plan()
    N, IN_C = x.shape
    E = edge_idx.shape[1]
    HEADS, HID = a_src1.shape
    HC = HEADS * HID
    OUT_C = W2.shape[1]
    B = math.ceil(N / (128 * n_cores))     # blocks per core
    NPC = B * 128                          # nodes per core
    NP = NPC * n_cores                     # padded node count
    assert half % 128 == 0 and half <= NP
    p.N, p.E, p.IN_C, p.HEADS, p.HID, p.HC, p.OUT_C = N, E, IN_C, HEADS, HID, HC, OUT_C
    p.B, p.NPC, p.NP, p.n_cores, p.HALF = B, NPC, NP, n_cores, half

    x_pad = np.zeros((NP, IN_C), np.float32)
    x_pad[:N] = x

    src = edge_idx[0].astype(np.int64)
    dst = edge_idx[1].astype(np.int64)
    order = np.argsort(dst, kind="stable")
    src, dst, attr = src[order], dst[order], edge_attr[order]

    # per (core, block, half) edge lists
    counts = np.zeros((n_cores, B, 2), np.int64)
    edges = [[[None, None] for _ in range(B)] for _ in range(n_cores)]
    blk = dst // 128
    blk_starts = np.searchsorted(blk, np.arange(NP // 128 + 1))
    for g in range(NP // 128):
        k, b = divmod(g, B)
        lo_, hi_ = blk_starts[g], blk_starts[g + 1]
        s_, d_, a_ = src[lo_:hi_], dst[lo_:hi_], attr[lo_:hi_]
        m = s_ < half
        edges[k][b][0] = (s_[m], d_[m] - g * 128, a_[m])
        edges[k][b][1] = (s_[~m] - half, d_[~m] - g * 128, a_[~m])
        counts[k, b, 0] = m.sum()
        counts[k, b, 1] = (~m).sum()

    cn = np.maximum(np.ceil(counts / CHUNK).astype(np.int64).max(axis=0), 0)
    p.cnA = cn[:, 0].tolist()   # lo chunks per block slot (uniform across cores)
    p.cnB = cn[:, 1].tolist()
    TC = int(sum(p.cnA) + sum(p.cnB))   # total chunks per core
    p.TC = TC

    gidx = np.zeros((n_cores, 16, TC * 8), np.int16)
    dstrel = np.zeros((n_cores, 128, TC), np.float32)
    maskattr = np.zeros((n_cores, 128, TC * 2), np.float32)
    for k in range(n_cores):
        c0 = 0
        for b in range(B):
            for hf in range(2):
                nch = (p.cnA[b], p.cnB[b])[hf]
                if nch == 0:
                    continue
                s_, dr_, a_ = edges[k][b][hf]
                ne = len(s_)
                nslots = nch * CHUNK
                sv = np.zeros(nslots, np.int16)
                dv = np.zeros(nslots, np.float32)
                av = np.zeros(nslots, np.float32)
                mv = np.zeros(nslots, np.float32)
                sv[:ne] = s_
                dv[:ne] = dr_
                av[:ne] = a_
                mv[:ne] = 1.0
                for c in range(nch):
                    sl = slice(c * CHUNK, (c + 1) * CHUNK)
                    cc = c0 + c
                    # gather slot i -> idx[i % 16, 8*chunk + i // 16]
                    gidx[k, :, cc * 8 : cc * 8 + 8] = sv[sl].reshape(8, 16).T
                    dstrel[k, :, cc] = dv[sl]
                    maskattr[k, :, cc * 2] = mv[sl]
                    maskattr[k, :, cc * 2 + 1] = av[sl]
                c0 += nch
        assert c0 == TC

    # weights packaging
    WaWd = np.zeros((HC, 2 * HEADS), np.float32)
    for h in range(HEADS):
        WaWd[h * HID : (h + 1) * HID, h] = a_src1[h]
        WaWd[h * HID : (h + 1) * HID, HEADS + h] = a_dst1[h]

    p.in_maps = []
    for k in range(n_cores):
        p.in_maps.append(
            {
                "x": x_pad,
                "x_own": x_pad[k * NPC : (k + 1) * NPC].copy(),
                "W1": W1,
                "WaWd": WaWd,
                "W2": W2,
                "b1row": b1[None, :],
                "b2row": b2[None, :],
                "We1row": We1,
                "ae1row": a_e1.reshape(1, HC),
                "We2row": We2,
                "ae2row": a_e2.reshape(1, OUT_C),
                "asrc2row": a_src2,
                "adst2row": a_dst2,
                "gidx": np.tile(gidx[k], (8, 1)),
                "dstrel": dstrel[k],
                "maskattr": maskattr[k],
            }
        )
    return p


def build(p, upto="full"):
    nc = bacc.Bacc(
        "TRN2",
        target_bir_lowering=False,
        debug=False,
        num_devices=p.n_cores,
        num_swdge_queues=4,
    )
    IN_C, HC, OUT_C, HEADS, HID = p.IN_C, p.HC, p.OUT_C, p.HEADS, p.HID
    NP, NPC, B, TC, HALF = p.NP, p.NPC, p.B, p.TC, p.HALF

    x_ext = nc.dram_tensor("x", [NP, IN_C], F32, kind="ExternalInput")
    xown_ext = nc.dram_tensor("x_own", [NPC, IN_C], F32, kind="ExternalInput")
    W1_ext = nc.dram_tensor("W1", [IN_C, HC], F32, kind="ExternalInput")
    WaWd_ext = nc.dram_tensor("WaWd", [HC, 2 * HEADS], F32, kind="ExternalInput")
    W2_ext = nc.dram_tensor("W2", [HC, OUT_C], F32, kind="ExternalInput")
    b1row_ext = nc.dram_tensor("b1row", [1, HC], F32, kind="ExternalInput")
    b2row_ext = nc.dram_tensor("b2row", [1, OUT_C], F32, kind="ExternalInput")
    We1row_ext = nc.dram_tensor("We1row", [1, HC], F32, kind="ExternalInput")
    ae1row_ext = nc.dram_tensor("ae1row", [1, HC], F32, kind="ExternalInput")
    We2row_ext = nc.dram_tensor("We2row", [1, OUT_C], F32, kind="ExternalInput")
    ae2row_ext = nc.dram_tensor("ae2row", [1, OUT_C], F32, kind="ExternalInput")
    asrc2row_ext = nc.dram_tensor("asrc2row", [1, OUT_C], F32, kind="ExternalInput")
    adst2row_ext = nc.dram_tensor("adst2row", [1, OUT_C], F32, kind="ExternalInput")
    gidx_ext = nc.dram_tensor("gidx", [128, TC * 8], I16, kind="ExternalInput")
    dstrel_ext = nc.dram_tensor("dstrel", [128, TC], F32, kind="ExternalInput")
    maskattr_ext = nc.dram_tensor("maskattr", [128, TC * 2], F32, kind="ExternalInput")
    out_ext = nc.dram_tensor("out", [NPC, OUT_C], F32, kind="ExternalOutput")

    table1 = nc.dram_tensor("table1", [NP, ELEM1], F32)
    own1 = nc.dram_tensor("own1", [NPC, IN_C + 8], F32)
    ag_in = nc.dram_tensor("ag_in", [NPC, ELEM2], F32)
    table2 = nc.dram_tensor("table2", [NP, ELEM2], F32, addr_space="Shared")

    C1 = IN_C           # table1 col: h1 at 0:HC(=128), asrc at 128:132, adst 132:136
    qn = [0]            # rotating gather queue

    def next_q():
        q = qn[0] % 4
        qn[0] += 1
        return q

    with tile.TileContext(nc) as tc:
        with (
            tc.tile_pool(name="consts", bufs=1) as cp,
            tc.tile_pool(name="streams", bufs=1) as strp,
        ):
            # ---------- constants & streams ----------
            ident = cp.tile([128, 128], F32)
            make_identity(nc, ident[:])
            iota_i = cp.tile([128, 128], I32)
            nc.gpsimd.iota(iota_i[:], pattern=[[1, 128]], base=0, channel_multiplier=0)
            iota_mat = cp.tile([128, 128], F32)
            nc.vector.tensor_copy(out=iota_mat[:], in_=iota_i[:])
            ones1 = cp.tile([1, 128], F32)
            nc.gpsimd.memset(ones1[:], 1.0)
            zeros_pad = cp.tile([128, ELEM2], F32)
            nc.gpsimd.memset(zeros_pad[:], 0.0)

            W1s = cp.tile([128, HC], F32)
            nc.sync.dma_start(out=W1s[:], in_=W1_ext[:, :])
            WaWds = cp.tile([128, 2 * HEADS], F32)
            nc.sync.dma_start(out=WaWds[:], in_=WaWd_ext[:, :])
            W2s = cp.tile([128, OUT_C], F32)
            nc.sync.dma_start(out=W2s[:], in_=W2_ext[:, :])

            rows = cp.tile([1, 128], F32, tag="rows")
            gidx_t = strp.tile([128, TC * 8], I16)
            nc.sync.dma_start(out=gidx_t[:], in_=gidx_ext[:, :])
            dstrel_t = strp.tile([128, TC], F32)
            nc.sync.dma_start(out=dstrel_t[:], in_=dstrel_ext[:, :])
            maskattr_t = strp.tile([128, TC * 2], F32)
            nc.sync.dma_start(out=maskattr_t[:], in_=maskattr_ext[:, :])

            la_keep = strp.tile([128, B], F32)

            # broadcast helper: row [1, n] -> [128, n] via ones-matmul
            with tc.tile_pool(name="bc_psum", bufs=2, space="PSUM") as bps:

                def bcast_row(row_ap, n, out_tile):
                    ps = bps.tile([128, 512], F32, tag="bc")
                    nc.tensor.matmul(
                        out=ps[:, :n], lhsT=ones1[:], rhs=row_ap, start=True, stop=True
                    )
                    nc.vector.tensor_copy(out=out_tile[:], in_=ps[:, :n])

                # c1[h] = sum_c We1[h*HID+c] * ae1[h*HID+c]; c_mat [128, HEADS]
                t_we = cp.tile([1, HC], F32, tag="t_we")
                nc.sync.dma_start(out=t_we[:], in_=We1row_ext[:, :])
                t_ae = cp.tile([1, HC], F32, tag="t_ae")
                nc.sync.dma_start(out=t_ae[:], in_=ae1row_ext[:, :])
                t_pr = cp.tile([1, HC], F32, tag="t_pr")
                nc.vector.tensor_mul(out=t_pr[:], in0=t_we[:], in1=t_ae[:])
                c1row = cp.tile([1, HEADS], F32, tag="c1row")
                nc.vector.tensor_reduce(
                    out=c1row[:],
                    in_=t_pr[:].rearrange("p (h c) -> p h c", c=HID),
                    axis=mybir.AxisListType.X,
                    op=OP.add,
                )
                c_mat = cp.tile([128, HEADS], F32)
                bcast_row(c1row[:], HEADS, c_mat)

                # c2 scalar -> c2col [128, 1]
                t_we2 = cp.tile([1, OUT_C], F32, tag="t_we2")
                nc.sync.dma_start(out=t_we2[:], in_=We2row_ext[:, :])
                t_ae2 = cp.tile([1, OUT_C], F32, tag="t_ae2")
                nc.sync.dma_start(out=t_ae2[:], in_=ae2row_ext[:, :])
                t_pr2 = cp.tile([1, OUT_C], F32, tag="t_pr2")
                nc.vector.tensor_mul(out=t_pr2[:], in0=t_we2[:], in1=t_ae2[:])
                c2row = cp.tile([1, 1], F32, tag="c2row")
                nc.vector.tensor_reduce(
                    out=c2row[:], in_=t_pr2[:], axis=mybir.AxisListType.X, op=OP.add
                )
                c2col = cp.tile([128, 1], F32)
                bcast_row(c2row[:], 1, c2col)

                b1mat = cp.tile([128, HC], F32)
                nc.sync.dma_start(out=rows[:, :HC], in_=b1row_ext[:, :])
                bcast_row(rows[:, :HC], HC, b1mat)
                b2mat = cp.tile([128, OUT_C], F32)
                rows2 = cp.tile([1, OUT_C], F32, tag="rows2")
                nc.sync.dma_start(out=rows2[:], in_=b2row_ext[:, :])
                bcast_row(rows2[:], OUT_C, b2mat)
                asrc2w = cp.tile([128, OUT_C], F32)
                rows3 = cp.tile([1, OUT_C], F32, tag="rows3")
                nc.sync.dma_start(out=rows3[:], in_=asrc2row_ext[:, :])
                bcast_row(rows3[:], OUT_C, asrc2w)
                adst2w = cp.tile([128, OUT_C], F32)
                rows4 = cp.tile([1, OUT_C], F32, tag="rows4")
                nc.sync.dma_start(out=rows4[:], in_=adst2row_ext[:, :])
                bcast_row(rows4[:], OUT_C, adst2w)

            # ---------- phase A: projection tables ----------
            def proj_rows(src_dram, g, dst_dram, dst_off):
                """rows [g*128, (g+1)*128) of src_dram -> table rows in dst_dram."""
                xa = pA["x"].tile([128, IN_C], F32, tag="xa")
                nc.sync.dma_start(
                    out=xa[:], in_=src_dram[g * 128 : (g + 1) * 128, :]
                )
                pt = pA["ps"].tile([128, 128], F32, space="PSUM", tag="pt")
                nc.tensor.transpose(out=pt[:, :IN_C], in_=xa[:], identity=ident[:])
                xT = pA["x"].tile([128, 128], F32, tag="xT")
                nc.vector.tensor_copy(out=xT[:, :], in_=pt[:, :])
                ph = pA["ps"].tile([128, HC], F32, space="PSUM", tag="ph")
                nc.tensor.matmul(
                    out=ph[:], lhsT=xT[:IN_C, :], rhs=W1s[:IN_C, :], start=True, stop=True
                )
                row_w = dst_dram.shape[1]
                row = pA["x"].tile([128, row_w], F32, tag="row")
                nc.vector.tensor_copy(out=row[:, 0:HC], in_=ph[:])
                pt2 = pA["ps"].tile([128, 128], F32, space="PSUM", tag="pt")
                nc.tensor.transpose(
                    out=pt2[:, :HC], in_=row[:, 0:HC], identity=ident[:]
                )
                h1T = pA["x"].tile([128, 128], F32, tag="h1T")
                nc.vector.tensor_copy(out=h1T[:], in_=pt2[:])
                paw = pA["ps"].tile([128, 2 * HEADS], F32, space="PSUM", tag="ph")
                nc.tensor.matmul(
                    out=paw[:], lhsT=h1T[:HC, :], rhs=WaWds[:HC, :], start=True, stop=True
                )
                nc.vector.tensor_copy(
                    out=row[:, HC : HC + 2 * HEADS], in_=paw[:]
                )
                if row_w > HC + 2 * HEADS:
                    nc.gpsimd.memset(row[:, HC + 2 * HEADS : row_w], 0.0)
                nc.sync.dma_start(
                    out=dst_dram[dst_off + g * 128 : dst_off + (g + 1) * 128, :],
                    in_=row[:],
                )

            with (
                tc.tile_pool(name="pA_x", bufs=3) as pA_x,
                tc.tile_pool(name="pA_ps", bufs=2, space="PSUM") as pA_ps,
            ):
                pA = {"x": pA_x, "ps": pA_ps}
                for g in range(NP // 128):
                    proj_rows(x_ext, g, table1, 0)
                for b in range(B):
                    proj_rows(xown_ext, b, own1, 0)
            _stages = {"A": 0, "L1": 1, "AG": 2, "full": 3}[upto]

            # ---------- shared per-layer machinery ----------
            def layer_pass(
                layer,
                table_dram,
                elem,
                nheads,
                msg_w,      # message width (= nheads * per-head channels)
                ob_dram,    # per-core own rows [NPC, *]: h|asrc|adst columns
                ob_cols,    # (h_lo, h_hi, asrc_lo, asrc_hi, adst_lo, adst_hi)
                cvec,       # [128, nheads] (c_mat or c2col)
                do_degattr, # accumulate deg/attr columns (layer 1)
                epilogue,   # fn(b, P, OB, exl) -> None
            ):
                hlo, hhi, alo, ahi, dlo, dhi = ob_cols
                rhs_w = msg_w + nheads + (2 if do_degattr else 0)
                c_glob = [0]
                with (
                    tc.tile_pool(name=f"g{layer}", bufs=3) as gp,
                    tc.tile_pool(name=f"rhs{layer}", bufs=3) as rhp,
                    tc.tile_pool(name=f"u{layer}", bufs=3) as upo,
                    tc.tile_pool(name=f"sel{layer}", bufs=4) as selp,
                    tc.tile_pool(name=f"sm{layer}", bufs=6) as smp,
                    tc.tile_pool(name=f"ob{layer}", bufs=2) as obp,
                    tc.tile_pool(name=f"ep{layer}", bufs=2) as epp,
                    tc.tile_pool(name=f"ps{layer}", bufs=2, space="PSUM") as psb,
                    tc.tile_pool(name=f"pst{layer}", bufs=2, space="PSUM") as pst,
                    tc.tile_pool(name=f"pss{layer}", bufs=2, space="PSUM") as pss,
                ):
                    for b in range(B):
                        OB = obp.tile([128, dhi], F32, tag="OB")
                        nc.sync.dma_start(
                            out=OB[:], in_=ob_dram[b * 128 : (b + 1) * 128, 0:dhi]
                        )
                        ncht = p.cnA[b] + p.cnB[b]
                        if ncht > 0:
                            pblk = psb.tile(
                                [128, rhs_w], F32, space="PSUM", tag="pblk"
                            )
                        cdone = 0
                        for hf in range(2):
                            nch = (p.cnA[b], p.cnB[b])[hf]
                            if nch == 0:
                                continue
                            tbl = (
                                table_dram[0:HALF, :]
                                if hf == 0
                                else table_dram[HALF:NP, :]
                            )
                            for c0 in range(0, nch, CALL_CHUNKS):
                                ncall = min(CALL_CHUNKS, nch - c0)
                                cg0 = c_glob[0] + cdone + c0
                                GA = gp.tile([128, CALL_CHUNKS * elem], F32, tag="GA")
                                nc.gpsimd.dma_gather(
                                    out_ap=GA[:, : ncall * elem].rearrange(
                                        "p (n k) -> p n k", k=elem
                                    ),
                                    in_ap=tbl,
                                    idxs_ap=gidx_t[:, cg0 * 8 : (cg0 + ncall) * 8],
                                    num_idxs=ncall * CHUNK,
                                    num_idxs_reg=ncall * CHUNK,
                                    elem_size=elem,
                                    queue_num=next_q(),
                                )
                                # u = attr*c + asrc_gathered  [128, ncall, nheads]
                                u = upo.tile(
                                    [128, CALL_CHUNKS * nheads], F32, tag="u"
                                )
                                u3 = u[:, : ncall * nheads].rearrange(
                                    "p (n k) -> p n k", k=nheads
                                )
                                attr_sl = _ap3(
                                    maskattr_t[:, cg0 * 2 + 1 : cg0 * 2 + 2],
                                    [
                                        list(maskattr_t[:].ap[0]),
                                        [2, ncall],
                                        [0, nheads],
                                    ],
                                )
                                nc.vector.tensor_tensor(
                                    out=u3,
                                    in0=attr_sl,
                                    in1=_bcast_mid(cvec[:], ncall),
                                    op=OP.mult,
                                )
                                ga_asrc0 = GA[:, msg_w : msg_w + nheads]
                                nc.vector.tensor_tensor(
                                    out=u3,
                                    in0=u3,
                                    in1=_ap3(
                                        ga_asrc0,
                                        [
                                            list(ga_asrc0.ap[0]),
                                            [elem, ncall],
                                            [1, nheads],
                                        ],
                                    ),
                                    op=OP.add,
                                )
                                rhs = rhp.tile(
                                    [128, CALL_CHUNKS * rhs_w], F32, tag="rhs"
                                )
                                if do_degattr:
                                    rh_ma0 = rhs[
                                        :, msg_w + nheads : msg_w + nheads + 2
                                    ]
                                    ma0 = maskattr_t[:, cg0 * 2 : cg0 * 2 + 2]
                                    nc.vector.tensor_copy(
                                        out=_ap3(
                                            rh_ma0,
                                            [
                                                list(rh_ma0.ap[0]),
                                                [rhs_w, ncall],
                                                [1, 2],
                                            ],
                                        ),
                                        in_=_ap3(
                                            ma0,
                                            [
                                                list(ma0.ap[0]),
                                                [2, ncall],
                                                [1, 2],
                                            ],
                                        ),
                                    )
                                for c in range(ncall):
                                    cg = cg0 + c
                                    is_first = cdone + c0 + c == 0
                                    is_last = cdone + c0 + c == ncht - 1
                                    S = selp.tile([128, 128], F32, tag="S")
                                    nc.vector.tensor_tensor(
                                        out=S[:],
                                        in0=iota_mat[:],
                                        in1=dstrel_t[:, cg : cg + 1].to_broadcast(
                                            [128, 128]
                                        ),
                                        op=OP.is_equal,
                                    )
                                    pT = pst.tile(
                                        [128, 128], F32, space="PSUM", tag="pT"
                                    )
                                    nc.tensor.transpose(
                                        out=pT[:], in_=S[:], identity=ident[:]
                                    )
                                    ST = selp.tile([128, 128], F32, tag="ST")
                                    nc.vector.tensor_copy(out=ST[:], in_=pT[:])
                                    padst = pss.tile(
                                        [128, nheads], F32, space="PSUM", tag="padst"
                                    )
                                    nc.tensor.matmul(
                                        out=padst[:],
                                        lhsT=ST[:],
                                        rhs=OB[:, dlo:dhi],
                                        start=True,
                                        stop=True,
                                    )
                                    lg = smp.tile([128, nheads], F32, tag="lg")
                                    nc.vector.tensor_tensor(
                                        out=lg[:],
                                        in0=u[:, c * nheads : (c + 1) * nheads],
                                        in1=padst[:],
                                        op=OP.add,
                                    )
                                    lk = smp.tile([128, nheads], F32, tag="lk")
                                    nc.vector.scalar_tensor_tensor(
                                        out=lk[:], in0=lg[:], scalar=NEG_SLOPE,
                                        in1=lg[:], op0=OP.mult, op1=OP.max,
                                    )
                                    exf = smp.tile([128, nheads], F32, tag="exf")
                                    nc.scalar.activation(
                                        out=exf[:], in_=lk[:], func=AF.Exp
                                    )
                                    ex_dst = rhs[
                                        :, c * rhs_w + msg_w : c * rhs_w + msg_w + nheads
                                    ]
                                    nc.vector.tensor_tensor(
                                        out=ex_dst,
                                        in0=exf[:],
                                        in1=maskattr_t[
                                            :, cg * 2 : cg * 2 + 1
                                        ].to_broadcast([128, nheads]),
                                        op=OP.mult,
                                    )
                                    hw = msg_w // nheads
                                    nc.vector.tensor_tensor(
                                        out=rhs[
                                            :, c * rhs_w : c * rhs_w + msg_w
                                        ].rearrange("p (h w) -> p h w", w=hw),
                                        in0=GA[
                                            :, c * elem : c * elem + msg_w
                                        ].rearrange("p (h w) -> p h w", w=hw),
                                        in1=_bcast_last(ex_dst, hw),
                                        op=OP.mult,
                                    )
                                    nc.tensor.matmul(
                                        out=pblk[:],
                                        lhsT=S[:],
                                        rhs=rhs[:, c * rhs_w : (c + 1) * rhs_w],
                                        start=is_first,
                                        stop=is_last,
                                    )
                            cdone += nch
                        c_glob[0] += ncht

                        # ----- block epilogue -----
                        P = epp.tile([128, rhs_w], F32, tag="P")
                        if ncht == 0:
                            nc.gpsimd.memset(P[:], 0.0)
                        else:
                            nc.vector.tensor_copy(out=P[:], in_=pblk[:])
                        # self-loop logits
                        if do_degattr:
                            dm = epp.tile([128, 1], F32, tag="dm")
                            nc.vector.tensor_scalar_max(
                                out=dm[:], in0=P[:, msg_w + nheads : msg_w + nheads + 1],
                                scalar1=1.0,
                            )
                            rc = epp.tile([128, 1], F32, tag="rc")
                            nc.vector.reciprocal(out=rc[:], in_=dm[:])
                            nc.vector.tensor_tensor(
                                out=la_keep[:, b : b + 1],
                                in0=P[:, msg_w + nheads + 1 : msg_w + nheads + 2],
                                in1=rc[:],
                                op=OP.mult,
                            )
                        tl = epp.tile([128, nheads], F32, tag="tl")
                        nc.vector.tensor_tensor(
                            out=tl[:],
                            in0=la_keep[:, b : b + 1].to_broadcast([128, nheads]),
                            in1=cvec[:],
                            op=OP.mult,
                        )
                        nc.vector.tensor_tensor(
                            out=tl[:], in0=tl[:], in1=OB[:, alo:ahi], op=OP.add
                        )
                        nc.vector.tensor_tensor(
                            out=tl[:], in0=tl[:], in1=OB[:, dlo:dhi], op=OP.add
                        )
                        lkl = epp.tile([128, nheads], F32, tag="lkl")
                        nc.vector.scalar_tensor_tensor(
                            out=lkl[:], in0=tl[:], scalar=NEG_SLOPE,
                            in1=tl[:], op0=OP.mult, op1=OP.max,
                        )
                        exl = epp.tile([128, nheads], F32, tag="exl")
                        nc.scalar.activation(out=exl[:], in_=lkl[:], func=AF.Exp)
                        hw = msg_w // nheads
                        ml = epp.tile([128, msg_w], F32, tag="ml")
                        nc.vector.tensor_tensor(
                            out=ml[:].rearrange("p (h w) -> p h w", w=hw),
                            in0=OB[:, hlo:hhi].rearrange("p (h w) -> p h w", w=hw),
                            in1=_bcast_last(exl[:], hw),
                            op=OP.mult,
                        )
                        tm = epp.tile([128, msg_w], F32, tag="tm")
                        nc.vector.tensor_tensor(
                            out=tm[:], in0=P[:, 0:msg_w], in1=ml[:], op=OP.add
                        )
                        st = epp.tile([128, nheads], F32, tag="st")
                        nc.vector.tensor_tensor(
                            out=st[:],
                            in0=P[:, msg_w : msg_w + nheads],
                            in1=exl[:],
                            op=OP.add,
                        )
                        rs = epp.tile([128, nheads], F32, tag="rs")
                        nc.vector.reciprocal(out=rs[:], in_=st[:])
                        o1 = epp.tile([128, msg_w], F32, tag="o1")
                        nc.vector.tensor_tensor(
                            out=o1[:].rearrange("p (h w) -> p h w", w=hw),
                            in0=tm[:].rearrange("p (h w) -> p h w", w=hw),
                            in1=_bcast_last(rs[:], hw),
                            op=OP.mult,
                        )
                        epilogue(b, o1)

            # ---------- layer 1 ----------
            if _stages >= 1:
             with (
                tc.tile_pool(name="e1", bufs=2) as e1p,
                tc.tile_pool(name="e1ps", bufs=1, space="PSUM") as e1ps,
            ):

                def epi1(b, o1):
                    # o1 = aggregated/normalized messages [128, HC] (pre-bias)
                    ob1 = e1p.tile([128, p.HC], F32, tag="ob1")
                    nc.vector.tensor_tensor(
                        out=ob1[:], in0=o1[:], in1=b1mat[:], op=OP.add
                    )
                    # elu
                    mn = e1p.tile([128, p.HC], F32, tag="mn")
                    nc.vector.tensor_scalar_min(out=mn[:], in0=ob1[:], scalar1=0.0)
                    em = e1p.tile([128, p.HC], F32, tag="em")
                    nc.scalar.activation(out=em[:], in_=mn[:], func=AF.Exp)
                    mx = e1p.tile([128, p.HC], F32, tag="mx")
                    nc.vector.tensor_scalar_max(out=mx[:], in0=ob1[:], scalar1=0.0)
                    x2 = e1p.tile([128, p.HC], F32, tag="x2")
                    nc.vector.tensor_add(out=x2[:], in0=mx[:], in1=em[:])
                    nc.vector.tensor_scalar_add(out=x2[:], in0=x2[:], scalar1=-1.0)
                    # h2 = x2 @ W2
                    pt = e1ps.tile([128, 128], F32, space="PSUM", tag="ept")
                    nc.tensor.transpose(out=pt[:, : p.HC], in_=x2[:], identity=ident[:])
                    x2T = e1p.tile([128, 128], F32, tag="x2T")
                    nc.vector.tensor_copy(out=x2T[:], in_=pt[:])
                    ph2 = e1ps.tile([128, p.OUT_C], F32, space="PSUM", tag="eph2")
                    nc.tensor.matmul(
                        out=ph2[:], lhsT=x2T[: p.HC, :], rhs=W2s[: p.HC, :],
                        start=True, stop=True,
                    )
                    h2s = e1p.tile([128, p.OUT_C], F32, tag="h2s")
                    nc.vector.tensor_copy(out=h2s[:], in_=ph2[:])
                    th = e1p.tile([128, p.OUT_C], F32, tag="th")
                    nc.vector.tensor_mul(out=th[:], in0=h2s[:], in1=asrc2w[:])
                    as2 = e1p.tile([128, 1], F32, tag="as2")
                    nc.vector.tensor_reduce(
                        out=as2[:], in_=th[:], axis=mybir.AxisListType.X, op=OP.add
                    )
                    nc.vector.tensor_mul(out=th[:], in0=h2s[:], in1=adst2w[:])
                    ad2 = e1p.tile([128, 1], F32, tag="ad2")
                    nc.vector.tensor_reduce(
                        out=ad2[:], in_=th[:], axis=mybir.AxisListType.X, op=OP.add
                    )
                    t2t = e1p.tile([128, ELEM2], F32, tag="t2t")
                    nc.vector.tensor_copy(out=t2t[:, : p.OUT_C], in_=h2s[:])
                    nc.vector.tensor_copy(
                        out=t2t[:, p.OUT_C : p.OUT_C + 1], in_=as2[:]
                    )
                    nc.vector.tensor_copy(
                        out=t2t[:, p.OUT_C + 1 : p.OUT_C + 2], in_=ad2[:]
                    )
                    nc.vector.tensor_copy(
                        out=t2t[:, p.OUT_C + 2 : ELEM2],
                        in_=zeros_pad[:, : ELEM2 - p.OUT_C - 2],
                    )
                    nc.sync.dma_start(
                        out=ag_in[b * 128 : (b + 1) * 128, :], in_=t2t[:]
                    )

                layer_pass(
                    layer=1,
                    table_dram=table1,
                    elem=ELEM1,
                    nheads=HEADS,
                    msg_w=HC,
                    ob_dram=own1,
                    ob_cols=(0, HC, HC, HC + HEADS, HC + HEADS, HC + 2 * HEADS),
                    cvec=c_mat,
                    do_degattr=True,
                    epilogue=epi1,
                )

            # ---------- allgather ----------
            if _stages >= 2:
                nc.gpsimd.collective_compute(
                    "AllGather",
                    OP.bypass,
                    replica_groups=[list(range(p.n_cores))],
                    ins=[ag_in.ap().opt()],
                    outs=[table2.ap().opt()],
                )

            # ---------- layer 2 ----------
            if _stages >= 3:
             with tc.tile_pool(name="e2", bufs=2) as e2p:

                def epi2(b, o2):
                    ob2 = e2p.tile([128, p.OUT_C], F32, tag="ob2")
                    nc.vector.tensor_tensor(
                        out=ob2[:], in0=o2[:], in1=b2mat[:], op=OP.add
                    )
                    nc.sync.dma_start(
                        out=out_ext[b * 128 : (b + 1) * 128, :], in_=ob2[:]
                    )

                layer_pass(
                    layer=2,
                    table_dram=table2,
                    elem=ELEM2,
                    nheads=1,
                    msg_w=OUT_C,
                    ob_dram=ag_in,
                    ob_cols=(0, OUT_C, OUT_C, OUT_C + 1, OUT_C + 1, OUT_C + 2),
                    cvec=c2col,
                    do_degattr=False,
                    epilogue=epi2,
                )

    nc.compile()
    return nc


def kernel(**inputs):
    p = host_prep(inputs)
    nc = build(p)
    from concourse.bass_utils import run_bass_kernel_spmd

    res = run_bass_kernel_spmd(nc, p.in_maps, list(range(p.n_cores))).results
    out = np.concatenate([res[k]["out"] for k in range(p.n_cores)], axis=0)
    return out[: p.N]


# revision 15
# speedup vs baseline: 2.8204x; 1.7918x over previous
"""Bass/Tile kernel for 2-layer edge-featured GAT (AblationGAT) on 8 trn2 cores.

Strategy (edge-parallel, dst-sharded):
  - Nodes padded to NP = n_cores * B * 128; core k owns blocks [k*B, (k+1)*B).
  - Host sorts edges by dst, assigns each edge to the core owning its dst
    block, splits per-block edges into lo/hi halves by src (int16 gather
    limit), pads each half to chunks of 128 edge slots (pad edges carry
    logit -inf via a -1e4 mask bias).
  - Phase A (per core, redundant): full projection tables
      table1[n] = [h1(128) | a_src1(4) | a_dst1(4) | pad]   (f32, 768B rows)
    computed transpose-free from host-provided xT; plus own1/adT1 for the
    core's own nodes at fixed local offsets.
  - Layer pass (per dst block): dma_gather table rows by src plus a second
    dma_gather of per-dst attention scalars (4 SWDGE queues), build
    per-edge softmax numerators via exp (no max subtraction needed:
    |logit| is small), scatter-add into a PSUM accumulator via one-hot
    selection matmuls. Self-loops (PyG fill_value='mean') handled in a
    per-block epilogue from deg / attr-sum columns of the same matmul.
  - AllGather of per-core layer-2 tables (h2 | a_src2 | a_dst2), then the
    same machinery for layer 2; output shard is the core's own node range.
"""

import math

import numpy as np

import concourse.bass as bass
import concourse.mybir as mybir
import concourse.tile as tile
from concourse import bacc
from concourse.masks import make_identity
from concourse.tile import add_dep_helper

F32 = mybir.dt.float32
I16 = mybir.dt.int16
I32 = mybir.dt.int32
AF = mybir.ActivationFunctionType
OP = mybir.AluOpType

NEG_SLOPE = 0.2
N_CORES = 8
CHUNK = 128          # edge slots per chunk (= PE contraction dim)
CALL_CHUNKS = 8      # chunks per dma_gather call (<= 1024 idx, HW limit)
ELEM1 = 192          # table1 row f32 elements (768B, %256B)
ELEM2 = 64           # table2 / adT row f32 elements (256B)
MASK_BIAS = -1.0e4   # pad-edge logit bias: exp(leaky(~-1e4)) == 0 in f32


def _ap3(ap, dims):
    """Raw AP with explicit [step, count] dims on the same tensor/offset."""
    return bass.AP(ap.tensor, ap.offset, [list(d) for d in dims])


def _bcast_last(ap, m):
    """[P, K] -> [P, K, m] with 0-stride last dim."""
    return _ap3(ap, list(ap.ap) + [[0, m]])


def _bcast_mid(ap, n):
    """[P, K] -> [P, n, K] with 0-stride middle dim."""
    return _ap3(ap, [list(ap.ap[0]), [0, n], list(ap.ap[-1])])


class Plan:
    pass


def host_prep(inputs, n_cores=N_CORES, half=32768):
    x = np.asarray(inputs["x"], np.float32)
    edge_idx = np.asarray(inputs["edge_idx"])
    edge_attr = np.asarray(inputs["edge_attr"], np.float32)[:, 0]
    W1 = np.asarray(inputs["W1"], np.float32)
    a_src1 = np.asarray(inputs["a_src1"], np.float32)
    a_dst1 = np.asarray(inputs["a_dst1"], np.float32)
    We1 = np.asarray(inputs["We1"], np.float32)
    a_e1 = np.asarray(inputs["a_e1"], np.float32)
    b1 = np.asarray(inputs["b1"], np.float32)
    W2 = np.asarray(inputs["W2"], np.float32)
    a_src2 = np.asarray(inputs["a_src2"], np.float32)
    a_dst2 = np.asarray(inputs["a_dst2"], np.float32)
    We2 = np.asarray(inputs["We2"], np.float32)
    a_e2 = np.asarray(inputs["a_e2"], np.float32)
    b2 = np.asarray(inputs["b2"], np.float32)

    p = Plan()
    N, IN_C = x.shape
    E = edge_idx.shape[1]
    HEADS, HID = a_src1.shape
    HC = HEADS * HID
    OUT_C = W2.shape[1]
    B = math.ceil(N / (128 * n_cores))     # blocks per core
    NPC = B * 128                          # nodes per core
    NP = NPC * n_cores                     # padded node count
    assert half % 128 == 0 and half <= NP
    p.N, p.E, p.IN_C, p.HEADS, p.HID, p.HC, p.OUT_C = N, E, IN_C, HEADS, HID, HC, OUT_C
    p.B, p.NPC, p.NP, p.n_cores, p.HALF = B, NPC, NP, n_cores, half

    x_pad = np.zeros((NP, IN_C), np.float32)
    x_pad[:N] = x
    xT = np.ascontiguousarray(x_pad.T)              # [IN_C, NP]

    src = edge_idx[0].astype(np.int64)
    dst = edge_idx[1].astype(np.int64)
    order = np.argsort(dst, kind="stable")
    src, dst, attr = src[order], dst[order], edge_attr[order]

    counts = np.zeros((n_cores, B, 2), np.int64)
    edges = [[[None, None] for _ in range(B)] for _ in range(n_cores)]
    blk = dst // 128
    blk_starts = np.searchsorted(blk, np.arange(NP // 128 + 1))
    for g in range(NP // 128):
        k, b = divmod(g, B)
        lo_, hi_ = blk_starts[g], blk_starts[g + 1]
        s_, d_, a_ = src[lo_:hi_], dst[lo_:hi_], attr[lo_:hi_]
        m = s_ < half
        edges[k][b][0] = (s_[m], d_[m] - g * 128, a_[m])
        edges[k][b][1] = (s_[~m] - half, d_[~m] - g * 128, a_[~m])
        counts[k, b, 0] = m.sum()
        counts[k, b, 1] = (~m).sum()

    cn = np.maximum(np.ceil(counts / CHUNK).astype(np.int64).max(axis=0), 0)
    p.cnA = cn[:, 0].tolist()
    p.cnB = cn[:, 1].tolist()
    TC = int(sum(p.cnA) + sum(p.cnB))
    p.TC = TC

    gidx = np.zeros((n_cores, 16, TC * 8), np.int16)
    didx = np.zeros((n_cores, 16, TC * 8), np.int16)   # local dst ids, adT gather
    dstrel = np.zeros((n_cores, 128, TC), np.float32)
    maskattr = np.zeros((n_cores, 128, TC * 2), np.float32)
    maskbias = np.zeros((n_cores, 128, TC), np.float32)
    for k in range(n_cores):
        c0 = 0
        for b in range(B):
            for hf in range(2):
                nch = (p.cnA[b], p.cnB[b])[hf]
                if nch == 0:
                    continue
                s_, dr_, a_ = edges[k][b][hf]
                ne = len(s_)
                nslots = nch * CHUNK
                sv = np.zeros(nslots, np.int16)
                dv = np.zeros(nslots, np.float32)
                av = np.zeros(nslots, np.float32)
                m01 = np.zeros(nslots, np.float32)
                sv[:ne] = s_
                dv[:ne] = dr_
                av[:ne] = a_
                m01[:ne] = 1.0
                lv = (b * 128 + dv).astype(np.int16)
                for c in range(nch):
                    sl = slice(c * CHUNK, (c + 1) * CHUNK)
                    cc = c0 + c
                    gidx[k, :, cc * 8 : cc * 8 + 8] = sv[sl].reshape(8, 16).T
                    didx[k, :, cc * 8 : cc * 8 + 8] = lv[sl].reshape(8, 16).T
                    dstrel[k, :, cc] = dv[sl]
                    maskattr[k, :, cc * 2] = m01[sl]
                    maskattr[k, :, cc * 2 + 1] = av[sl]
                    maskbias[k, :, cc] = (1.0 - m01[sl]) * MASK_BIAS
                c0 += nch
        assert c0 == TC

    WaWd = np.zeros((HC, 2 * HEADS), np.float32)
    for h in range(HEADS):
        WaWd[h * HID : (h + 1) * HID, h] = a_src1[h]
        WaWd[h * HID : (h + 1) * HID, HEADS + h] = a_dst1[h]

    p.in_maps = []
    for k in range(n_cores):
        p.in_maps.append(
            {
                "xT": xT,
                "xTown": np.ascontiguousarray(xT[:, k * NPC : (k + 1) * NPC]),
                "W1": W1,
                "WaWd": WaWd,
                "W2": W2,
                "b1row": b1[None, :],
                "b2row": b2[None, :],
                "We1row": We1,
                "ae1row": a_e1.reshape(1, HC),
                "We2row": We2,
                "ae2row": a_e2.reshape(1, OUT_C),
                "asrc2row": a_src2,
                "adst2row": a_dst2,
                "gidx": np.tile(gidx[k], (8, 1)),
                "didx": np.tile(didx[k], (8, 1)),
                "dstrel": dstrel[k],
                "maskattr": maskattr[k],
                "maskbias": maskbias[k],
            }
        )
    return p


def build(p, upto="full"):
    nc = bacc.Bacc(
        "TRN2",
        target_bir_lowering=False,
        debug=False,
        num_devices=p.n_cores,
        num_swdge_queues=4,
    )
    IN_C, HC, OUT_C, HEADS, HID = p.IN_C, p.HC, p.OUT_C, p.HEADS, p.HID
    NP, NPC, B, TC, HALF = p.NP, p.NPC, p.B, p.TC, p.HALF

    xT_ext = nc.dram_tensor("xT", [IN_C, NP], F32, kind="ExternalInput")
    xTown_ext = nc.dram_tensor("xTown", [IN_C, NPC], F32, kind="ExternalInput")
    W1_ext = nc.dram_tensor("W1", [IN_C, HC], F32, kind="ExternalInput")
    WaWd_ext = nc.dram_tensor("WaWd", [HC, 2 * HEADS], F32, kind="ExternalInput")
    W2_ext = nc.dram_tensor("W2", [HC, OUT_C], F32, kind="ExternalInput")
    b1row_ext = nc.dram_tensor("b1row", [1, HC], F32, kind="ExternalInput")
    b2row_ext = nc.dram_tensor("b2row", [1, OUT_C], F32, kind="ExternalInput")
    We1row_ext = nc.dram_tensor("We1row", [1, HC], F32, kind="ExternalInput")
    ae1row_ext = nc.dram_tensor("ae1row", [1, HC], F32, kind="ExternalInput")
    We2row_ext = nc.dram_tensor("We2row", [1, OUT_C], F32, kind="ExternalInput")
    ae2row_ext = nc.dram_tensor("ae2row", [1, OUT_C], F32, kind="ExternalInput")
    asrc2row_ext = nc.dram_tensor("asrc2row", [1, OUT_C], F32, kind="ExternalInput")
    adst2row_ext = nc.dram_tensor("adst2row", [1, OUT_C], F32, kind="ExternalInput")
    gidx_ext = nc.dram_tensor("gidx", [128, TC * 8], I16, kind="ExternalInput")
    didx_ext = nc.dram_tensor("didx", [128, TC * 8], I16, kind="ExternalInput")
    dstrel_ext = nc.dram_tensor("dstrel", [128, TC], F32, kind="ExternalInput")
    maskattr_ext = nc.dram_tensor("maskattr", [128, TC * 2], F32, kind="ExternalInput")
    maskbias_ext = nc.dram_tensor("maskbias", [128, TC], F32, kind="ExternalInput")
    out_ext = nc.dram_tensor("out", [NPC, OUT_C], F32, kind="ExternalOutput")

    table1 = nc.dram_tensor("table1", [NP, ELEM1], F32)
    own1 = nc.dram_tensor("own1", [NPC, IN_C + 8], F32)
    adT1 = nc.dram_tensor("adT1", [NPC, ELEM2], F32)
    ag_in = nc.dram_tensor("ag_in", [NPC, ELEM2], F32)
    table2 = nc.dram_tensor("table2", [NP, ELEM2], F32, addr_space="Shared")

    qn = [0]
    last_g = [None]

    def next_q():
        q = qn[0] % 4
        qn[0] += 1
        return q

    def chain_g(inst):
        # Keep Pool DMA instructions in program order so Tile's DMASW sem
        # lane rotation (scheduled order) matches our queue_num rotation.
        if last_g[0] is not None:
            add_dep_helper(inst.ins, last_g[0], sync=False,
                           reason="swdge queue/lane alignment")
        last_g[0] = inst.ins

    GRP = 4  # phase-A chunks per DMA group

    with tile.TileContext(nc) as tc:
        with (
            tc.tile_pool(name="consts", bufs=1) as cp,
            tc.tile_pool(name="streams", bufs=1) as strp,
        ):
            # ---------- constants & streams ----------
            ident = cp.tile([128, 128], F32)
            make_identity(nc, ident[:])
            iota_i = cp.tile([128, 128], I32)
            nc.gpsimd.iota(iota_i[:], pattern=[[1, 128]], base=0, channel_multiplier=0)
            iota_mat = cp.tile([128, 128], F32)
            nc.vector.tensor_copy(out=iota_mat[:], in_=iota_i[:])
            ones1 = cp.tile([1, 128], F32)
            nc.gpsimd.memset(ones1[:], 1.0)
            zeros_pad = cp.tile([128, ELEM2], F32)
            nc.gpsimd.memset(zeros_pad[:], 0.0)

            W1s = cp.tile([128, HC], F32)
            nc.sync.dma_start(out=W1s[:], in_=W1_ext[:, :])
            WaWds = cp.tile([128, 2 * HEADS], F32)
            nc.sync.dma_start(out=WaWds[:], in_=WaWd_ext[:, :])
            Vw = cp.tile([128, 2 * HEADS], F32)  # W1 @ WaWd  [IN_C, 2H]
            W2s = cp.tile([128, OUT_C], F32)
            nc.sync.dma_start(out=W2s[:], in_=W2_ext[:, :])

            rows = cp.tile([1, 128], F32, tag="rows")
            gidx_t = strp.tile([128, TC * 8], I16)
            nc.sync.dma_start(out=gidx_t[:], in_=gidx_ext[:, :])
            didx_t = strp.tile([128, TC * 8], I16)
            nc.sync.dma_start(out=didx_t[:], in_=didx_ext[:, :])
            dstrel_t = strp.tile([128, TC], F32)
            nc.sync.dma_start(out=dstrel_t[:], in_=dstrel_ext[:, :])
            maskattr_t = strp.tile([128, TC * 2], F32)
            nc.sync.dma_start(out=maskattr_t[:], in_=maskattr_ext[:, :])
            maskbias_t = strp.tile([128, TC], F32)
            nc.sync.dma_start(out=maskbias_t[:], in_=maskbias_ext[:, :])

            la_keep = strp.tile([128, B], F32)

            with tc.tile_pool(name="bc_psum", bufs=2, space="PSUM") as bps:

                def bcast_row(row_ap, n, out_tile):
                    ps = bps.tile([128, 512], F32, tag="bc")
                    nc.tensor.matmul(
                        out=ps[:, :n], lhsT=ones1[:], rhs=row_ap, start=True, stop=True
                    )
                    nc.vector.tensor_copy(out=out_tile[:], in_=ps[:, :n])

                t_we = cp.tile([1, HC], F32, tag="t_we")
                nc.sync.dma_start(out=t_we[:], in_=We1row_ext[:, :])
                t_ae = cp.tile([1, HC], F32, tag="t_ae")
                nc.sync.dma_start(out=t_ae[:], in_=ae1row_ext[:, :])
                t_pr = cp.tile([1, HC], F32, tag="t_pr")
                nc.vector.tensor_mul(out=t_pr[:], in0=t_we[:], in1=t_ae[:])
                c1row = cp.tile([1, HEADS], F32, tag="c1row")
                nc.vector.tensor_reduce(
                    out=c1row[:],
                    in_=t_pr[:].rearrange("p (h c) -> p h c", c=HID),
                    axis=mybir.AxisListType.X,
                    op=OP.add,
                )
                c_mat = cp.tile([128, HEADS], F32)
                bcast_row(c1row[:], HEADS, c_mat)

                t_we2 = cp.tile([1, OUT_C], F32, tag="t_we2")
                nc.sync.dma_start(out=t_we2[:], in_=We2row_ext[:, :])
                t_ae2 = cp.tile([1, OUT_C], F32, tag="t_ae2")
                nc.sync.dma_start(out=t_ae2[:], in_=ae2row_ext[:, :])
                t_pr2 = cp.tile([1, OUT_C], F32, tag="t_pr2")
                nc.vector.tensor_mul(out=t_pr2[:], in0=t_we2[:], in1=t_ae2[:])
                c2row = cp.tile([1, 1], F32, tag="c2row")
                nc.vector.tensor_reduce(
                    out=c2row[:], in_=t_pr2[:], axis=mybir.AxisListType.X, op=OP.add
                )
                c2col = cp.tile([128, 1], F32)
                bcast_row(c2row[:], 1, c2col)

                # Vw = W1 @ WaWd via PE transpose of W1
                pw = bps.tile([128, 128], F32, tag="pw", space="PSUM")
                nc.tensor.transpose(out=pw[:, :HC], in_=W1s[:IN_C, :], identity=ident[:])
                W1Ts = cp.tile([128, 128], F32)
                nc.vector.tensor_copy(out=W1Ts[:], in_=pw[:])
                pv = bps.tile([128, 2 * HEADS], F32, tag="pv", space="PSUM")
                nc.tensor.matmul(
                    out=pv[:IN_C, :], lhsT=W1Ts[:HC, :IN_C], rhs=WaWds[:HC, :],
                    start=True, stop=True,
                )
                nc.vector.tensor_copy(out=Vw[:], in_=pv[:])

                b1mat = cp.tile([128, HC], F32)
                nc.sync.dma_start(out=rows[:, :HC], in_=b1row_ext[:, :])
                bcast_row(rows[:, :HC], HC, b1mat)
                b2mat = cp.tile([128, OUT_C], F32)
                rows2 = cp.tile([1, OUT_C], F32, tag="rows2")
                nc.sync.dma_start(out=rows2[:], in_=b2row_ext[:, :])
                bcast_row(rows2[:], OUT_C, b2mat)
                asrc2w = cp.tile([128, OUT_C], F32)
                rows3 = cp.tile([1, OUT_C], F32, tag="rows3")
                nc.sync.dma_start(out=rows3[:], in_=asrc2row_ext[:, :])
                bcast_row(rows3[:], OUT_C, asrc2w)
                adst2w = cp.tile([128, OUT_C], F32)
                rows4 = cp.tile([1, OUT_C], F32, tag="rows4")
                nc.sync.dma_start(out=rows4[:], in_=adst2row_ext[:, :])
                bcast_row(rows4[:], OUT_C, adst2w)

            # ---------- phase A (transpose-free) ----------
            def proj_group(src_dram, g4, nchunks, dst_dram, adT_dram=None):
                n_nodes = nchunks * 128
                base = g4 * (GRP * 128)
                row_w = dst_dram.shape[1]
                xTg = pA_x.tile([128, GRP * 128], F32, tag="xTg")
                nc.sync.dma_start(
                    out=xTg[:, :n_nodes], in_=src_dram[:, base : base + n_nodes]
                )
                rowg = pA_x.tile([128, GRP * ELEM1], F32, tag="rowg")
                adg = None
                if adT_dram is not None:
                    adg = pA_x.tile([128, GRP * ELEM2], F32, tag="adg")
                for j in range(nchunks):
                    xTc = xTg[:, j * 128 : (j + 1) * 128]
                    ph = pA_ps.tile([128, HC], F32, space="PSUM", tag="ph")
                    nc.tensor.matmul(
                        out=ph[:], lhsT=xTc, rhs=W1s[:IN_C, :], start=True, stop=True
                    )
                    rj = j * row_w
                    nc.vector.tensor_copy(out=rowg[:, rj : rj + HC], in_=ph[:])
                    paw = pA_ps.tile([128, 2 * HEADS], F32, space="PSUM", tag="paw")
                    nc.tensor.matmul(
                        out=paw[:], lhsT=xTc, rhs=Vw[:IN_C, :],
                        start=True, stop=True,
                    )
                    nc.vector.tensor_copy(
                        out=rowg[:, rj + HC : rj + HC + 2 * HEADS], in_=paw[:]
                    )
                    pad_w = row_w - (HC + 2 * HEADS)
                    if pad_w > 0:
                        nc.vector.tensor_copy(
                            out=rowg[:, rj + HC + 2 * HEADS : rj + row_w],
                            in_=_bcast_last(zeros_pad[:, :1], pad_w)
                            if pad_w > ELEM2
                            else zeros_pad[:, :pad_w],
                        )
                    if adg is not None:
                        aj = j * ELEM2
                        nc.vector.tensor_copy(
                            out=adg[:, aj : aj + HEADS],
                            in_=paw[:, HEADS : 2 * HEADS],
                        )
                        nc.vector.tensor_copy(
                            out=adg[:, aj + HEADS : aj + ELEM2],
                            in_=zeros_pad[:, : ELEM2 - HEADS],
                        )
                nc.sync.dma_start(
                    out=_ap3(
                        dst_dram[base : base + n_nodes, :],
                        [[row_w, 128], [row_w * 128, nchunks], [1, row_w]],
                    ),
                    in_=rowg[:, : nchunks * row_w].rearrange(
                        "p (n k) -> p n k", k=row_w
                    ),
                )
                if adg is not None:
                    nc.sync.dma_start(
                        out=_ap3(
                            adT_dram[base : base + n_nodes, :],
                            [[ELEM2, 128], [ELEM2 * 128, nchunks], [1, ELEM2]],
                        ),
                        in_=adg[:, : nchunks * ELEM2].rearrange(
                            "p (n k) -> p n k", k=ELEM2
                        ),
                    )

            with (
                tc.tile_pool(name="pA_x", bufs=3) as pA_x,
                tc.tile_pool(name="pA_ps", bufs=2, space="PSUM") as pA_ps,
            ):
                ng = NP // 128
                for g4 in range((ng + GRP - 1) // GRP):
                    proj_group(xT_ext, g4, min(GRP, ng - g4 * GRP), table1)
                ngo = NPC // 128
                for g4 in range((ngo + GRP - 1) // GRP):
                    proj_group(
                        xTown_ext, g4, min(GRP, ngo - g4 * GRP), own1, adT_dram=adT1
                    )

            _stages = {"A": 0, "L1": 1, "AG": 2, "full": 3}[upto]

            # ---------- shared per-layer machinery ----------
            def layer_pass(
                layer, table_dram, adT_dram, adT_col, elem, nheads, msg_w,
                ob_dram, ob_cols, cvec, do_degattr, epilogue,
                selp, obp, epp, gp, rhp, upo, psb,
            ):
                hlo, hhi, alo, ahi, dlo, dhi = ob_cols
                rhs_w = msg_w + nheads + (2 if do_degattr else 0)
                c_glob = 0
                for b in range(B):
                    OB = obp.tile([128, dhi], F32, tag="OB")
                    nc.sync.dma_start(
                        out=OB[:], in_=ob_dram[b * 128 : (b + 1) * 128, 0:dhi]
                    )
                    ncht = p.cnA[b] + p.cnB[b]
                    if ncht > 0:
                        pblk = psb.tile([128, rhs_w], F32, space="PSUM", tag="pblk")
                    cdone = 0
                    for hf in range(2):
                        nch = (p.cnA[b], p.cnB[b])[hf]
                        if nch == 0:
                            continue
                        tbl = (
                            table_dram[0:HALF, :]
                            if hf == 0
                            else table_dram[HALF:NP, :]
                        )
                        for c0 in range(0, nch, CALL_CHUNKS):
                            ncall = min(CALL_CHUNKS, nch - c0)
                            cg0 = c_glob + cdone + c0
                            ni = ncall * CHUNK
                            GA = gp.tile([128, CALL_CHUNKS * elem], F32, tag="GA")
                            _g1 = nc.gpsimd.dma_gather(
                                out_ap=GA[:, : ncall * elem].rearrange(
                                    "p (n k) -> p n k", k=elem
                                ),
                                in_ap=tbl,
                                idxs_ap=gidx_t[:, cg0 * 8 : (cg0 + ncall) * 8],
                                num_idxs=ni,
                                num_idxs_reg=ni,
                                elem_size=elem,
                                queue_num=next_q(),
                            )
                            chain_g(_g1)
                            AD = gp.tile([128, CALL_CHUNKS * ELEM2], F32, tag="AD")
                            _g2 = nc.gpsimd.dma_gather(
                                out_ap=AD[:, : ncall * ELEM2].rearrange(
                                    "p (n k) -> p n k", k=ELEM2
                                ),
                                in_ap=adT_dram[:, :],
                                idxs_ap=didx_t[:, cg0 * 8 : (cg0 + ncall) * 8],
                                num_idxs=ni,
                                num_idxs_reg=ni,
                                elem_size=ELEM2,
                                queue_num=next_q(),
                            )
                            chain_g(_g2)
                            # u = attr*c + asrc + adst + maskbias  [128,ncall,nh]
                            u = upo.tile([128, CALL_CHUNKS * nheads], F32, tag="u")
                            u3 = u[:, : ncall * nheads].rearrange(
                                "p (n k) -> p n k", k=nheads
                            )
                            ma1 = maskattr_t[:, cg0 * 2 + 1 : cg0 * 2 + 2]
                            nc.vector.tensor_tensor(
                                out=u3,
                                in0=_ap3(
                                    ma1, [list(ma1.ap[0]), [2, ncall], [0, nheads]]
                                ),
                                in1=_bcast_mid(cvec[:], ncall),
                                op=OP.mult,
                            )
                            ga0 = GA[:, msg_w : msg_w + nheads]
                            nc.vector.tensor_tensor(
                                out=u3, in0=u3,
                                in1=_ap3(
                                    ga0, [list(ga0.ap[0]), [elem, ncall], [1, nheads]]
                                ),
                                op=OP.add,
                            )
                            ad0 = AD[:, adT_col : adT_col + nheads]
                            nc.vector.tensor_tensor(
                                out=u3, in0=u3,
                                in1=_ap3(
                                    ad0,
                                    [list(ad0.ap[0]), [ELEM2, ncall], [1, nheads]],
                                ),
                                op=OP.add,
                            )
                            mb0 = maskbias_t[:, cg0 : cg0 + 1]
                            nc.vector.tensor_tensor(
                                out=u3, in0=u3,
                                in1=_ap3(
                                    mb0, [list(mb0.ap[0]), [1, ncall], [0, nheads]]
                                ),
                                op=OP.add,
                            )
                            lk = upo.tile([128, CALL_CHUNKS * nheads], F32, tag="lk")
                            nc.vector.scalar_tensor_tensor(
                                out=lk[:, : ncall * nheads],
                                in0=u[:, : ncall * nheads],
                                scalar=NEG_SLOPE,
                                in1=u[:, : ncall * nheads],
                                op0=OP.mult, op1=OP.max,
                            )
                            rhs = rhp.tile(
                                [128, CALL_CHUNKS * rhs_w], F32, tag="rhs"
                            )
                            rhs_ex0 = rhs[:, msg_w : msg_w + nheads]
                            nc.scalar.activation(
                                out=_ap3(
                                    rhs_ex0,
                                    [list(rhs_ex0.ap[0]), [rhs_w, ncall], [1, nheads]],
                                ),
                                in_=lk[:, : ncall * nheads].rearrange(
                                    "p (n k) -> p n k", k=nheads
                                ),
                                func=AF.Exp,
                            )
                            if do_degattr:
                                rh_ma0 = rhs[:, msg_w + nheads : msg_w + nheads + 2]
                                ma0 = maskattr_t[:, cg0 * 2 : cg0 * 2 + 2]
                                nc.vector.tensor_copy(
                                    out=_ap3(
                                        rh_ma0,
                                        [list(rh_ma0.ap[0]), [rhs_w, ncall], [1, 2]],
                                    ),
                                    in_=_ap3(
                                        ma0, [list(ma0.ap[0]), [2, ncall], [1, 2]]
                                    ),
                                )
                            hw = msg_w // nheads
                            for c in range(ncall):
                                cg = cg0 + c
                                is_first = cdone + c0 + c == 0
                                is_last = cdone + c0 + c == ncht - 1
                                S = selp.tile([128, 128], F32, tag="S")
                                nc.vector.tensor_tensor(
                                    out=S[:],
                                    in0=iota_mat[:],
                                    in1=dstrel_t[:, cg : cg + 1].to_broadcast(
                                        [128, 128]
                                    ),
                                    op=OP.is_equal,
                                )
                                nc.vector.tensor_tensor(
                                    out=rhs[
                                        :, c * rhs_w : c * rhs_w + msg_w
                                    ].rearrange("p (h w) -> p h w", w=hw),
                                    in0=GA[
                                        :, c * elem : c * elem + msg_w
                                    ].rearrange("p (h w) -> p h w", w=hw),
                                    in1=_bcast_last(
                                        rhs[
                                            :,
                                            c * rhs_w + msg_w : c * rhs_w
                                            + msg_w + nheads,
                                        ],
                                        hw,
                                    ),
                                    op=OP.mult,
                                )
                                nc.tensor.matmul(
                                    out=pblk[:],
                                    lhsT=S[:],
                                    rhs=rhs[:, c * rhs_w : (c + 1) * rhs_w],
                                    start=is_first,
                                    stop=is_last,
                                )
                        cdone += nch
                    c_glob += ncht

                    # ----- block epilogue (reads pblk PSUM directly) -----
                    assert ncht > 0
                    if do_degattr:
                        dm = epp.tile([128, 1], F32, tag="dm")
                        nc.vector.tensor_scalar_max(
                            out=dm[:],
                            in0=pblk[:, msg_w + nheads : msg_w + nheads + 1],
                            scalar1=1.0,
                        )
                        rc = epp.tile([128, 1], F32, tag="rc")
                        nc.vector.reciprocal(out=rc[:], in_=dm[:])
                        nc.vector.tensor_tensor(
                            out=la_keep[:, b : b + 1],
                            in0=pblk[:, msg_w + nheads + 1 : msg_w + nheads + 2],
                            in1=rc[:],
                            op=OP.mult,
                        )
                    # tl = la * c + asrc_own + adst_own   (2 fused ops)
                    tl = epp.tile([128, nheads], F32, tag="tl")
                    nc.vector.scalar_tensor_tensor(
                        out=tl[:], in0=cvec[:], scalar=la_keep[:, b : b + 1],
                        in1=OB[:, alo:ahi], op0=OP.mult, op1=OP.add,
                    )
                    nc.vector.tensor_tensor(
                        out=tl[:], in0=tl[:], in1=OB[:, dlo:dhi], op=OP.add
                    )
                    lkl = epp.tile([128, nheads], F32, tag="lkl")
                    nc.vector.scalar_tensor_tensor(
                        out=lkl[:], in0=tl[:], scalar=NEG_SLOPE, in1=tl[:],
                        op0=OP.mult, op1=OP.max,
                    )
                    exl = epp.tile([128, nheads], F32, tag="exl")
                    nc.scalar.activation(out=exl[:], in_=lkl[:], func=AF.Exp)
                    hw = msg_w // nheads
                    tm = epp.tile([128, msg_w], F32, tag="tm")
                    if nheads == 1:
                        # tm = OB_h * exl + pblk_msgs   (1 fused op)
                        nc.vector.scalar_tensor_tensor(
                            out=tm[:], in0=OB[:, hlo:hhi], scalar=exl[:],
                            in1=pblk[:, 0:msg_w], op0=OP.mult, op1=OP.add,
                        )
                    else:
                        ml = epp.tile([128, msg_w], F32, tag="ml")
                        nc.vector.tensor_tensor(
                            out=ml[:].rearrange("p (h w) -> p h w", w=hw),
                            in0=OB[:, hlo:hhi].rearrange("p (h w) -> p h w", w=hw),
                            in1=_bcast_last(exl[:], hw),
                            op=OP.mult,
                        )
                        nc.vector.tensor_tensor(
                            out=tm[:], in0=pblk[:, 0:msg_w], in1=ml[:], op=OP.add
                        )
                    st = epp.tile([128, nheads], F32, tag="st")
                    nc.vector.tensor_tensor(
                        out=st[:], in0=pblk[:, msg_w : msg_w + nheads], in1=exl[:],
                        op=OP.add,
                    )
                    rs = epp.tile([128, nheads], F32, tag="rs")
                    nc.vector.reciprocal(out=rs[:], in_=st[:])
                    o1 = epp.tile([128, msg_w], F32, tag="o1")
                    nc.vector.tensor_tensor(
                        out=o1[:].rearrange("p (h w) -> p h w", w=hw),
                        in0=tm[:].rearrange("p (h w) -> p h w", w=hw),
                        in1=_bcast_last(rs[:], hw),
                        op=OP.mult,
                    )
                    epilogue(b, o1)

            # ---------- layer 1 ----------
            if _stages >= 1:
             with (
                tc.tile_pool(name="g1", bufs=3) as gp,
                tc.tile_pool(name="rhs1", bufs=3) as rhp,
                tc.tile_pool(name="u1", bufs=3) as upo,
                tc.tile_pool(name="sel1", bufs=6) as selp,
                tc.tile_pool(name="ob1p", bufs=2) as obp,
                tc.tile_pool(name="ep1", bufs=2) as epp,
                tc.tile_pool(name="ps1", bufs=2, space="PSUM") as psb,
                tc.tile_pool(name="e1", bufs=2) as e1p,
                tc.tile_pool(name="e1ps", bufs=2, space="PSUM") as e1ps,
             ):

                def epi1(b, o1):
                    ob1 = e1p.tile([128, p.HC], F32, tag="ob1")
                    nc.vector.tensor_tensor(
                        out=ob1[:], in0=o1[:], in1=b1mat[:], op=OP.add
                    )
                    # elu(x) = max(x,0) + exp(min(x,0)) - 1   (3 DVE + 1 ACT)
                    mn = e1p.tile([128, p.HC], F32, tag="mn")
                    nc.vector.tensor_scalar_min(out=mn[:], in0=ob1[:], scalar1=0.0)
                    em = e1p.tile([128, p.HC], F32, tag="em")
                    nc.scalar.activation(out=em[:], in_=mn[:], func=AF.Exp)
                    x2 = e1p.tile([128, p.HC], F32, tag="x2")
                    nc.vector.scalar_tensor_tensor(
                        out=x2[:], in0=ob1[:], scalar=0.0, in1=em[:],
                        op0=OP.max, op1=OP.add,
                    )
                    nc.vector.tensor_scalar_add(out=x2[:], in0=x2[:], scalar1=-1.0)
                    pt = e1ps.tile([128, 128], F32, space="PSUM", tag="ept")
                    nc.tensor.transpose(out=pt[:, : p.HC], in_=x2[:], identity=ident[:])
                    x2T = e1p.tile([128, 128], F32, tag="x2T")
                    nc.vector.tensor_copy(out=x2T[:], in_=pt[:])
                    ph2 = e1ps.tile([128, p.OUT_C], F32, space="PSUM", tag="eph2")
                    nc.tensor.matmul(
                        out=ph2[:], lhsT=x2T[: p.HC, :], rhs=W2s[: p.HC, :],
                        start=True, stop=True,
                    )
                    t2t = e1p.tile([128, ELEM2], F32, tag="t2t")
                    nc.vector.tensor_copy(out=t2t[:, : p.OUT_C], in_=ph2[:])
                    th = e1p.tile([128, p.OUT_C], F32, tag="th")
                    nc.vector.tensor_mul(
                        out=th[:], in0=t2t[:, : p.OUT_C], in1=asrc2w[:]
                    )
                    nc.vector.tensor_reduce(
                        out=t2t[:, p.OUT_C : p.OUT_C + 1], in_=th[:],
                        axis=mybir.AxisListType.X, op=OP.add,
                    )
                    nc.vector.tensor_mul(
                        out=th[:], in0=t2t[:, : p.OUT_C], in1=adst2w[:]
                    )
                    nc.vector.tensor_reduce(
                        out=t2t[:, p.OUT_C + 1 : p.OUT_C + 2], in_=th[:],
                        axis=mybir.AxisListType.X, op=OP.add,
                    )
                    nc.vector.tensor_copy(
                        out=t2t[:, p.OUT_C + 2 : ELEM2],
                        in_=zeros_pad[:, : ELEM2 - p.OUT_C - 2],
                    )
                    nc.sync.dma_start(
                        out=ag_in[b * 128 : (b + 1) * 128, :], in_=t2t[:]
                    )

                layer_pass(
                    1, table1, adT1, 0, ELEM1, HEADS, HC, own1,
                    (0, HC, HC, HC + HEADS, HC + HEADS, HC + 2 * HEADS),
                    c_mat, True, epi1,
                    selp, obp, epp, gp, rhp, upo, psb,
                )

            # ---------- allgather ----------
            if _stages >= 2:
                nc.gpsimd.collective_compute(
                    "AllGather",
                    OP.bypass,
                    replica_groups=[list(range(p.n_cores))],
                    ins=[ag_in.ap().opt()],
                    outs=[table2.ap().opt()],
                )

            # ---------- layer 2 ----------
            if _stages >= 3:
             with (
                tc.tile_pool(name="g2", bufs=3) as gp2,
                tc.tile_pool(name="rhs2", bufs=3) as rhp2,
                tc.tile_pool(name="u2", bufs=3) as upo2,
                tc.tile_pool(name="sel2", bufs=6) as selp2,
                tc.tile_pool(name="ob2p", bufs=2) as obp2,
                tc.tile_pool(name="ep2", bufs=2) as epp2,
                tc.tile_pool(name="ps2", bufs=2, space="PSUM") as psb2,
                tc.tile_pool(name="e2", bufs=2) as e2p,
             ):

                def epi2(b, o2):
                    ob2 = e2p.tile([128, p.OUT_C], F32, tag="ob2")
                    nc.vector.tensor_tensor(
                        out=ob2[:], in0=o2[:], in1=b2mat[:], op=OP.add
                    )
                    nc.sync.dma_start(
                        out=out_ext[b * 128 : (b + 1) * 128, :], in_=ob2[:]
                    )

                layer_pass(
                    2, table2, ag_in, OUT_C + 1, ELEM2, 1, OUT_C, ag_in,
                    (0, OUT_C, OUT_C, OUT_C + 1, OUT_C + 1, OUT_C + 2),
                    c2col, False, epi2,
                    selp2, obp2, epp2, gp2, rhp2, upo2, psb2,
                )

    nc.compile()
    return nc


def kernel(**inputs):
    p = host_prep(inputs)
    nc = build(p)
    from concourse.bass_utils import run_bass_kernel_spmd

    res = run_bass_kernel_spmd(nc, p.in_maps, list(range(p.n_cores))).results
    out = np.concatenate([res[k]["out"] for k in range(p.n_cores)], axis=0)
    return out[: p.N]
